# revision 24
# baseline (speedup 1.0000x reference)
"""Trainium2 Bass kernel for nn_BaselineModelWithGNN (8-core SPMD).

Self-contained: hardcodes shapes/sharding; builds, compiles and runs the Bass
program on 8 NeuronCores via the axon PJRT path.

Key observation: the reference applies each of the 3 GCN convs to the same
input x and overwrites `out`, so only conv i=2 (w_conv[2], b_conv[2],
bng[4]) affects the result — one conv is computed.

Sharding: nodes (and their incident edges, dst-sharded) are partitioned
contiguously across the 8 cores (4096 nodes / 8 graphs per core); the
PLM/pooling branch is data-parallel over batch. BatchNorm statistics are
exchanged with a small AllGather + local combine; the 384-wide node
features y are AllGathered (fp16) for the edge gather.

Perf notes (CoreSim cost model):
- selector one-hots use fp16 iota/dstloc so TensorScalarPtr hits the 4x DVE
  mode (all-2-byte operands).
- BN statistics via the native bn_stats instruction (one pass, sum+var).
- dma_gather chunks span dst-window boundaries (16 tiles = 2048 descriptors
  per call; dynamic_dma_scratch_size=65536 gives a 4096-descriptor SWDGE
  ring) to amortize the ~1us fixed SWDGE overhead per gather.
- stat exchange = AllGather (15.6us) + 7 local adds, cheaper than AllReduce
  (28.2us) under the collective cost model.
"""
import sys
sys.path.insert(0, "/opt/trn_rl_repo")
from contextlib import ExitStack

import numpy as np
import ml_dtypes

import bass_rust as _br
import concourse.bacc as bacc
import concourse.bass as bass
import concourse.tile as tile
from concourse import mybir
from concourse._compat import cdiv

fp32 = mybir.dt.float32
bf16 = mybir.dt.float16  # "bf16" name kept; fp16 for 8x less quant noise
fp8 = mybir.dt.float8e4
i16 = mybir.dt.int16
AF = mybir.ActivationFunctionType
ALU = mybir.AluOpType
AX = mybir.AxisListType

NCORES = 8
B, S, E = 64, 512, 768
D = 384
NG = 512
N = B * NG              # 32768
NEDGE = 1048576
C = 3
NPC = N // NCORES       # 4096 nodes per core
GPC = B // NCORES       # 8 graphs per core
NW = NPC // 128         # 32 dst windows per core
NCOL = 512
NCH = NPC // NCOL       # 8 column chunks
EPS = 1e-5
GCHUNK = 8              # gather chunk: tiles (of 128 edges) per dma_gather;
                        # 1024 descs = the SWDGE ring; bigger wedges real HW
DPAD = 512              # y-table row bytes (fp8, padded): >=512B avoids the
                        # 2x DMA latency multiplier for sub-512B descriptors


# ---------------------------------------------------------------- BIR patch
def split_waits(nc):
    """walrus here supports ONE sync-wait per instruction; split extras onto
    NoOps inserted just before, on the same engine."""
    counter = 0
    for f in nc.m.functions:
        for bb in f.blocks:
            newlist, changed = [], False
            for inst in bb.instructions:
                si = inst.sync_info
                if si is not None and len(si.on_wait) > 1:
                    waits = list(si.on_wait)
                    for w in waits[:-1]:
                        counter += 1
                        nop = mybir.InstNoOp(name=f"I-WSPLIT-{counter}", ins=[], outs=[])
                        nop.engine = inst.engine
                        nop.sync_info = _br.SyncInfo(on_wait=[w], on_update=[])
                        newlist.append(nop)
                    inst.sync_info = _br.SyncInfo(
                        on_wait=[waits[-1]], on_update=list(si.on_update))
                    changed = True
                newlist.append(inst)
            if changed:
                bb.instructions = newlist


# ---------------------------------------------------------------- host prep
def _col3(v):
    """[384] -> [128, 3] column layout (feature f = c*128+p)."""
    return np.ascontiguousarray(v.reshape(3, 128).T).astype(np.float32)


def _col6(v):
    return np.ascontiguousarray(v.reshape(6, 128).T).astype(np.float32)


def _wchunks(w, kc, m):
    """[K, M] -> [128, kc, M] (k-chunk on partitions)."""
    K, M = w.shape
    assert K == kc * 128
    return np.ascontiguousarray(w.reshape(kc, 128, M).transpose(1, 0, 2))


def _wrap_idx(idx):
    """int16 idx array (len % 128 == 0) -> [128, len/16] dma_gather layout."""
    blk = idx.reshape(-1, 16).T  # [16, len/16]
    return np.ascontiguousarray(np.tile(blk, (8, 1)))


def preprocess(inputs):
    ei = np.asarray(inputs["edge_index"]).astype(np.int64)
    loop = np.arange(N, dtype=np.int64)
    src_all = np.concatenate([ei[0], loop])
    dst_all = np.concatenate([ei[1], loop])

    deg = np.bincount(dst_all, minlength=N).astype(np.float32)  # CSR row lengths

    win = dst_all >> 7
    order = np.lexsort((src_all, win))
    src_s = src_all[order]
    dst_s = dst_all[order]

    wcnt = np.bincount(win, minlength=N // 128)          # [256]
    tpw = np.maximum(
        np.ceil(wcnt.reshape(NCORES, NW) / 128).max(axis=0), 1
    ).astype(np.int64)                                    # [32] shared schedule
    T_total = int(tpw.sum())
    wstart = np.zeros(N // 128 + 1, np.int64)
    np.cumsum(wcnt, out=wstart[1:])
    tstart = np.zeros(NW + 1, np.int64)
    np.cumsum(tpw, out=tstart[1:])

    # masked node indices (2 per graph, ascending)
    mask = np.asarray(inputs["graph_masking"])
    sel = np.argsort(-mask, axis=1, kind="stable")[:, :2]  # top_k: ones, asc idx
    sel = np.sort(sel, axis=1)

    xT = np.zeros((D, N), np.float32)
    xT[:300] = np.asarray(inputs["x_nodes"]).T
    xT = xT.astype(np.float16)

    w1p = np.zeros((D, D), np.float32)
    w1p[:300] = np.asarray(inputs["w_pre1"])

    lastf = np.asarray(inputs["last_h"]).astype(np.float16)
    firstf = np.asarray(inputs["first_h"]).astype(np.float16)

    bng_g, bng_b = np.asarray(inputs["bng_g"]), np.asarray(inputs["bng_b"])
    bn_g, bn_b = np.asarray(inputs["bn_g"]), np.asarray(inputs["bn_b"])
    # vec columns [128, 72]: order documented here, mirrored on device
    cols = [
        _col3(np.asarray(inputs["b_pre1"])), _col3(np.asarray(inputs["b_pre2"])),
        _col3(np.asarray(inputs["b_post1"])), _col3(np.asarray(inputs["b_post2"])),
        _col6(np.asarray(inputs["b_cat"])),
        _col3(bng_g[0]), _col3(bng_b[0]), _col3(bng_g[1]), _col3(bng_b[1]),
        _col3(bng_g[4]), _col3(bng_b[4]), _col3(bng_g[5]), _col3(bng_b[5]),
        _col3(bng_g[6]), _col3(bng_b[6]),
        _col6(bn_g[0]), _col6(bn_b[0]), _col6(bn_g[1]), _col6(bn_b[1]),
    ]
    vecs = np.concatenate(cols, axis=1)  # [128, 3*4+6+3*10+6*4] = [128, 72]
    brow = np.zeros((1, 512), np.float32)
    brow[0, :D] = np.asarray(inputs["b_conv"])[2]
    brow[0, D:D + C] = np.asarray(inputs["b_out"])

    w_bf = {
        "w1": _wchunks(w1p, 3, D).astype(np.float16),
        "w2": _wchunks(np.asarray(inputs["w_pre2"]), 3, D).astype(np.float16),
        "wc": _wchunks(np.asarray(inputs["w_conv"])[2], 3, D).astype(np.float16),
        "wp1": _wchunks(np.asarray(inputs["w_post1"]), 3, D).astype(np.float16),
        "wp2": _wchunks(np.asarray(inputs["w_post2"]), 3, D).astype(np.float16),
    }
    wcat = _wchunks(np.asarray(inputs["w_cat"]), 6, E).astype(np.float16)
    wout = _wchunks(np.asarray(inputs["w_out"]), 6, C).astype(np.float32)

    in_maps = []
    for c in range(NCORES):
        n0 = c * NPC
        src_pad = np.zeros(T_total * 128, np.int64)
        dstloc = np.full(T_total * 128, -1.0, np.float32)
        for w in range(NW):
            gw = c * NW + w
            a, b_ = wstart[gw], wstart[gw + 1]
            k = b_ - a
            pos = tstart[w] * 128
            src_pad[pos:pos + k] = src_s[a:b_]
            dstloc[pos:pos + k] = (dst_s[a:b_] - gw * 128).astype(np.float32)
        # int16: N-1 = 32767 fits exactly
        idx_w = _wrap_idx(src_pad.astype(np.int16))            # [128, T*8]
        dst_t = np.ascontiguousarray(dstloc.reshape(T_total, 128).T)  # [128, T]

        deg_nm = np.ascontiguousarray(
            deg[n0:n0 + NPC].reshape(NW, 128).T)               # [128, 32]

        gidx = (sel[c * GPC:(c + 1) * GPC] +
                np.arange(c * GPC, (c + 1) * GPC)[:, None] * NG - n0)  # local
        gidx = gidx.reshape(-1).astype(np.int16)               # [16]
        gidx_w = np.zeros((128, 1), np.int16)
        gidx_w[:16, 0] = gidx
        gidx_w = np.tile(gidx_w[:16], (8, 1))

        m = {
            "lasth": np.ascontiguousarray(
                lastf[c * GPC:(c + 1) * GPC].reshape(GPC * S, E)),
            "firsth": np.ascontiguousarray(
                firstf[c * GPC:(c + 1) * GPC].reshape(GPC * S, E)),
            "xT": np.ascontiguousarray(
                xT.reshape(3, 128, N)[:, :, n0:n0 + NPC].transpose(1, 0, 2)
            ).reshape(128, 3 * NPC),
            "eidx": idx_w, "dstloc": dst_t, "deg": deg_nm,
            "vecs": vecs, "brow": brow, "gidx": gidx_w,
            "w1": w_bf["w1"].reshape(128, 3 * D),
            "w2": w_bf["w2"].reshape(128, 3 * D),
            "wc": w_bf["wc"].reshape(128, 3 * D),
            "wp1": w_bf["wp1"].reshape(128, 3 * D),
            "wp2": w_bf["wp2"].reshape(128, 3 * D),
            "wcat": wcat.reshape(128, 6 * E),
            "wout": wout.reshape(128, 6 * C),
        }
        in_maps.append(m)
    meta = (tuple(int(t) for t in tpw),)
    return in_maps, meta


# ---------------------------------------------------------------- device
def build(meta, rep=1, taps=(), stage=99):
    tpw = meta[0]
    T_total = sum(tpw)
    tstart = [0]
    for t in tpw:
        tstart.append(tstart[-1] + t)
    win_of = np.zeros(T_total, np.int64)
    for w in range(NW):
        win_of[tstart[w]:tstart[w + 1]] = w

    nc = bacc.Bacc("TRN2")
    I = {}
    I["lasth"] = nc.dram_tensor("lasth", [GPC * S, E], bf16, kind="ExternalInput")
    I["firsth"] = nc.dram_tensor("firsth", [GPC * S, E], bf16, kind="ExternalInput")
    I["xT"] = nc.dram_tensor("xT", [128, 3 * NPC], bf16, kind="ExternalInput")
    I["eidx"] = nc.dram_tensor("eidx", [128, T_total * 8], i16, kind="ExternalInput")
    I["dstloc"] = nc.dram_tensor("dstloc", [128, T_total], fp32, kind="ExternalInput")
    I["deg"] = nc.dram_tensor("deg", [128, NW], fp32, kind="ExternalInput")
    I["vecs"] = nc.dram_tensor("vecs", [128, 72], fp32, kind="ExternalInput")
    I["brow"] = nc.dram_tensor("brow", [1, 512], fp32, kind="ExternalInput")
    I["gidx"] = nc.dram_tensor("gidx", [128, 1], i16, kind="ExternalInput")
    for w in ("w1", "w2", "wc", "wp1", "wp2"):
        I[w] = nc.dram_tensor(w, [128, 3 * D], bf16, kind="ExternalInput")
    I["wcat"] = nc.dram_tensor("wcat", [128, 6 * E], bf16, kind="ExternalInput")
    I["wout"] = nc.dram_tensor("wout", [128, 6 * C], fp32, kind="ExternalInput")
    outT = nc.dram_tensor("outT", [C, GPC], fp32, kind="ExternalOutput")
    tap_outs = {}

    grp = [list(range(NCORES))]

    with tile.TileContext(nc) as tc, ExitStack() as ctx:
        const = ctx.enter_context(tc.tile_pool(name="const", bufs=1))
        big = ctx.enter_context(tc.tile_pool(name="big", bufs=2))
        gpool = ctx.enter_context(tc.tile_pool(name="gath", bufs=2))
        spool = ctx.enter_context(tc.tile_pool(name="small", bufs=2))
        selp = ctx.enter_context(tc.tile_pool(name="sel", bufs=8))
        hspool = ctx.enter_context(tc.tile_pool(name="hs", bufs=8))
        mmps = ctx.enter_context(tc.tile_pool(name="mmps", bufs=2, space="PSUM"))
        cvps = ctx.enter_context(tc.tile_pool(name="cvps", bufs=2, space="PSUM"))
        trps = ctx.enter_context(tc.tile_pool(name="trps", bufs=2, space="PSUM"))
        typs = ctx.enter_context(tc.tile_pool(name="typs", bufs=2, space="PSUM"))
        dram = ctx.enter_context(tc.tile_pool(name="dram", bufs=1, space="DRAM"))

        # ---------------- constants
        iota = const.tile([128, 128], bf16)
        nc.gpsimd.iota(iota[:], pattern=[[1, 128]], base=0, channel_multiplier=0,
                       allow_small_or_imprecise_dtypes=True)
        pidx = const.tile([128, 1], fp32)  # partition index column
        nc.gpsimd.iota(pidx[:], pattern=[[0, 1]], base=0, channel_multiplier=1,
                       allow_small_or_imprecise_dtypes=True)
        ident = const.tile([128, 128], bf16)
        nc.vector.tensor_scalar(ident[:], iota[:], pidx[:], None, ALU.is_equal)

        idx_t = const.tile([128, T_total * 8], i16)
        nc.sync.dma_start(idx_t[:], I["eidx"][:])
        dst_t = const.tile([128, T_total], fp32)
        nc.sync.dma_start(dst_t[:], I["dstloc"][:])
        deg_t = const.tile([128, NW], fp32)
        nc.sync.dma_start(deg_t[:], I["deg"][:])
        vecs = const.tile([128, 72], fp32)
        nc.sync.dma_start(vecs[:], I["vecs"][:])
        brow = const.tile([1, 512], fp32)
        nc.sync.dma_start(brow[:], I["brow"][:])
        gidx_t = const.tile([128, 1], i16)
        nc.sync.dma_start(gidx_t[:], I["gidx"][:])
        W = {}
        for w in ("w1", "w2", "wc", "wp1", "wp2"):
            W[w] = const.tile([128, 3, D], bf16, name=f"W_{w}", tag=f"W_{w}")
            nc.sync.dma_start(W[w][:], I[w][:].rearrange("p (k m) -> p k m", k=3))
        wcat = const.tile([128, 6, E], bf16)
        nc.sync.dma_start(wcat[:], I["wcat"][:].rearrange("p (k m) -> p k m", k=6))
        wout = const.tile([128, 6, C], fp32)
        nc.sync.dma_start(wout[:], I["wout"][:].rearrange("p (k m) -> p k m", k=6))
        bconv_bc = const.tile([128, D], fp32)
        nc.gpsimd.partition_broadcast(bconv_bc[:], brow[0:1, 0:D])
        ones8 = const.tile([1, GPC], fp32)
        nc.vector.memset(ones8[:], 1.0)
        onescol = const.tile([128, 1], bf16)
        nc.vector.memset(onescol[:], 1.0)

        # vec column offsets
        VO = {}
        off = 0
        for name, w_ in [("b1", 3), ("b2", 3), ("bp1", 3), ("bp2", 3), ("bcat", 6),
                         ("g0", 3), ("be0", 3), ("g1", 3), ("be1", 3),
                         ("g4", 3), ("be4", 3), ("g5", 3), ("be5", 3),
                         ("g6", 3), ("be6", 3),
                         ("gc0", 6), ("bc0", 6), ("gc1", 6), ("bc1", 6)]:
            VO[name] = (off, w_)
            off += w_
        def vcol(name):
            o, w_ = VO[name]
            return vecs[:, o:o + w_]

        # deg^-1/2
        dinv = const.tile([128, NW], fp32)
        nc.scalar.sqrt(dinv[:], deg_t[:])
        nc.vector.reciprocal(dinv[:], dinv[:])

        def tap(name, ap):
            if name not in taps:
                return
            t_ = nc.dram_tensor(f"tap_{name}", list(ap.shape), ap.dtype,
                                kind="ExternalOutput")
            tap_outs[name] = t_
            nc.sync.dma_start(t_[:], ap)

        def finish_early(src_ap, width=GPC):
            fin0 = spool.tile([C, GPC], fp32, name="fin0", tag="fin")
            nc.vector.memset(fin0[:], 0.0)
            nc.vector.tensor_scalar(fin0[:, 0:width], src_ap, 1.0, None, ALU.mult)
            nc.sync.dma_start(outT[:], fin0[:])

        for _rep in range(rep):
            # ---------------- helpers
            def _combine_bn_raw(raw, st, m_chunks):
                """raw [128, m, NCH, 6] bn_stats outputs -> st [128, m, 2]
                holding (sum/256, sumsq/256) per feature."""
                Ev = raw[:, :, :, 1]
                Vv = raw[:, :, :, 2]
                Ov = raw[:, :, :, 4]
                Wv = raw[:, :, :, 5]
                e2 = spool.tile([128, m_chunks, NCH], fp32, name="e2", tag="cb_e2")
                o2 = spool.tile([128, m_chunks, NCH], fp32, name="o2", tag="cb_o2")
                vv = spool.tile([128, m_chunks, NCH], fp32, name="vv", tag="cb_vv")
                ss = spool.tile([128, m_chunks, NCH], fp32, name="ss", tag="cb_ss")
                nc.vector.tensor_mul(e2[:], Ev, Ev)
                nc.vector.tensor_mul(o2[:], Ov, Ov)
                nc.vector.tensor_add(e2[:], e2[:], o2[:])
                nc.vector.tensor_tensor(vv[:], Vv, Wv, ALU.add)
                nc.vector.scalar_tensor_tensor(
                    vv[:], vv[:], 1.0 / 256, e2[:], op0=ALU.mult, op1=ALU.add)
                nc.vector.tensor_tensor(ss[:], Ev, Ov, ALU.add)
                nc.vector.reduce_sum(st[:, :, 0:1], ss[:], axis=AX.X)
                nc.vector.reduce_sum(st[:, :, 1:2], vv[:], axis=AX.X)

            def bn_stats_pack(src_view, m_chunks, tag):
                """src_view(m) -> [128, NCH, NCOL] bf16 view; returns
                [128, m_chunks*2] (sum/256, sumsq/256) stat tile."""
                raw = spool.tile([128, m_chunks, NCH, 6], fp32, tag=tag + "_bs")
                for m in range(m_chunks):
                    sv = src_view(m)
                    for j in range(NCH):
                        nc.vector.bn_stats(raw[:, m, j:j + 1, :], sv[:, j, :])
                st = spool.tile([128, m_chunks, 2], fp32, tag=tag + "_st")
                _combine_bn_raw(raw, st, m_chunks)
                return st

            def ag_exchange(st_ap, width, tag):
                """AllGather the [128, width] stat tile; sum over ranks."""
                cin = dram.tile([128, width], fp32, tag=tag + "_ci")
                cout = dram.tile([NCORES * 128, width], fp32, tag=tag + "_co")
                nc.sync.dma_start(cin[:], st_ap)
                nc.gpsimd.collective_compute(
                    "AllGather", ALU.bypass, replica_groups=grp,
                    ins=[cin[:]], outs=[cout[:]])
                allsb = spool.tile([128, NCORES, width], fp32, tag=tag + "_as")
                nc.sync.dma_start(
                    allsb[:], cout[:].rearrange("(r p) w -> p r w", p=128))
                red = spool.tile([128, width], fp32, tag=tag + "_rd")
                nc.vector.tensor_tensor(red[:], allsb[:, 0], allsb[:, 1], ALU.add)
                for r in range(2, NCORES):
                    nc.vector.tensor_tensor(red[:], red[:], allsb[:, r], ALU.add)
                return red

            def bn_coeffs(red, m_chunks, count, gname, bname, tag):
                """red [128, 2*m]: per-feature sum/sumsq -> gp, bp [128, m]"""
                gp = spool.tile([128, m_chunks], fp32, tag=tag + "_gp")
                bp = spool.tile([128, m_chunks], fp32, tag=tag + "_bp")
                mu = spool.tile([128, m_chunks], fp32, tag=tag + "_mu")
                var = spool.tile([128, m_chunks], fp32, tag=tag + "_va")
                inv_n = 1.0 / count
                sview = red.rearrange("p (m two) -> p m two", two=2)
                nc.vector.tensor_scalar(mu[:], sview[:, :, 0], inv_n, None, ALU.mult)
                nc.vector.tensor_scalar(var[:], sview[:, :, 1], inv_n, None, ALU.mult)
                musq = spool.tile([128, m_chunks], fp32, tag=tag + "_ms")
                nc.vector.tensor_mul(musq[:], mu[:], mu[:])
                nc.vector.tensor_tensor(var[:], var[:], musq[:], ALU.subtract)
                nc.vector.tensor_scalar(var[:], var[:], EPS, None, ALU.add)
                nc.scalar.sqrt(var[:], var[:])
                nc.vector.reciprocal(var[:], var[:])          # 1/sigma
                nc.vector.tensor_mul(gp[:], vcol(gname), var[:])
                nc.vector.tensor_mul(bp[:], gp[:], mu[:])
                nc.vector.tensor_tensor(bp[:], vcol(bname), bp[:], ALU.subtract)
                return gp, bp

            def bn_apply(x_t, gp, bp, m_chunks, width):
                for m in range(m_chunks):
                    nc.vector.tensor_scalar(
                        x_t[:, m, :width], x_t[:, m, :width],
                        gp[:, m:m + 1], bp[:, m:m + 1], ALU.mult, ALU.add)

            def mlp_layer(x_t, w_t, bias_col, tag):
                """x_t [128,3,NPC] bf16 -> relu(x@W + b) bf16 [128,3,NPC] + stats"""
                out = big.tile([128, 3, NPC], bf16, tag="big")
                for m in range(3):
                    for j in range(NCH):
                        sl = slice(j * NCOL, (j + 1) * NCOL)
                        ps = mmps.tile([128, NCOL], fp32, tag="mm")
                        for k in range(3):
                            nc.tensor.matmul(
                                ps[:], lhsT=w_t[:, k, m * 128:(m + 1) * 128],
                                rhs=x_t[:, k, sl], start=(k == 0), stop=(k == 2))
                        nc.scalar.activation(out[:, m, sl], ps[:], AF.Relu,
                                             bias=bias_col[:, m:m + 1])
                st = bn_stats_pack(
                    lambda m: out[:, m, :].rearrange("p (j n) -> p j n", n=NCOL),
                    3, tag)
                return out, st

            # ---------------- pre-MLPs
            xT_t = big.tile([128, 3, NPC], bf16, tag="big")
            nc.sync.dma_start(xT_t[:], I["xT"][:].rearrange("p (k n) -> p k n", k=3))

            x1, st1 = mlp_layer(xT_t, W["w1"], vcol("b1"), "l1")
            red1 = ag_exchange(st1[:], 6, "ar1")
            gp1, bp1 = bn_coeffs(red1, 3, N // 256, "g0", "be0", "bn1")
            bn_apply(x1, gp1, bp1, 3, NPC)
            tap("x1", x1[:])

            x2, st2 = mlp_layer(x1, W["w2"], vcol("b2"), "l2")
            red2 = ag_exchange(st2[:], 6, "ar2")
            gp2, bp2 = bn_coeffs(red2, 3, N // 256, "g1", "be1", "bn2")
            bn_apply(x2, gp2, bp2, 3, NPC)
            tap("x2", x2[:])
            if stage <= 2:
                finish_early(x2[0:C, 0, 0:GPC])
                continue

            # ---------------- z = x2 @ wc ; y = z * dinv (node-major, fp8
            # rows padded to DPAD bytes so each gather descriptor is >=512B)
            y_slice = dram.tile([NPC, D], bf16, tag="y_slice")
            z_sb = big.tile([128, 3, NPC], bf16, tag="big")
            for m in range(3):
                for j in range(NCH):
                    sl = slice(j * NCOL, (j + 1) * NCOL)
                    ps = mmps.tile([128, NCOL], fp32, tag="mm")
                    for k in range(3):
                        nc.tensor.matmul(
                            ps[:], lhsT=W["wc"][:, k, m * 128:(m + 1) * 128],
                            rhs=x2[:, k, sl], start=(k == 0), stop=(k == 2))
                    nc.scalar.activation(z_sb[:, m, sl], ps[:], AF.Copy)
            for w in range(NW):
                ywin = spool.tile([128, D], bf16, tag="ywin")
                for m in range(3):
                    trp = trps.tile([128, 128], bf16, tag="tr")
                    nc.tensor.transpose(
                        trp[:], z_sb[:, m, w * 128:(w + 1) * 128], ident[:])
                    nc.vector.tensor_scalar(
                        ywin[:, m * 128:(m + 1) * 128], trp[:],
                        dinv[:, w:w + 1], None, ALU.mult)
                nc.sync.dma_start(y_slice[w * 128:(w + 1) * 128, :], ywin[:])

            y_full = dram.tile([N, D], bf16, tag="y_full", addr_space="Shared")
            nc.gpsimd.collective_compute(
                "AllGather", ALU.bypass, replica_groups=grp,
                ins=[y_slice[:]], outs=[y_full[:]])

            # ---------------- sentence branch (overlaps the y AllGather)
            HsT = spool.tile([128, 6, GPC], fp32, tag="HsT")
            for b in range(GPC):
                ps_ht = typs.tile([128, GPC], fp32, tag="tiny")
                hts = []
                for hsrc in (I["lasth"], I["firsth"]):
                    for sc in range(4):
                        ht = hspool.tile([128, E], bf16, name="ht", tag="ht")
                        nc.sync.dma_start(
                            ht[:], hsrc[b * S + sc * 128:b * S + (sc + 1) * 128, :])
                        hts.append(ht)
                for m in range(6):
                    for i, ht in enumerate(hts):
                        nc.tensor.matmul(
                            ps_ht[:, m:m + 1],
                            lhsT=ht[:, m * 128:(m + 1) * 128],
                            rhs=onescol[:],
                            start=(i == 0), stop=(i == 7))
                nc.vector.tensor_scalar(
                    HsT[:, :, b], ps_ht[:, 0:6],
                    1.0 / (2 * S), None, ALU.mult)
            tap("hsT", HsT[:])
            if stage <= 3:
                yck = spool.tile([128, D], bf16, name="yck", tag="ywin")
                nc.sync.dma_start(yck[:], y_full[0:128, :])
                finish_early(yck[0:C, 0:GPC])
                continue

            # ---------------- conv: window-spanning gather chunks + selector
            # matmuls accumulating per-dst-window psum
            convT = big.tile([128, 3, NPC], bf16, tag="big")
            ps_win = {}

            def finalize_window(w):
                ps_c = ps_win.pop(w)
                winf = spool.tile([128, D], fp32, tag="winf")
                nc.vector.scalar_tensor_tensor(
                    winf[:], ps_c[:], dinv[:, w:w + 1], bconv_bc[:],
                    op0=ALU.mult, op1=ALU.add)
                winb = spool.tile([128, D], bf16, tag="winb")
                nc.scalar.activation(winb[:], winf[:], AF.Relu)
                for m in range(3):
                    trp = trps.tile([128, 128], bf16, tag="tr")
                    nc.tensor.transpose(trp[:], winb[:, m * 128:(m + 1) * 128],
                                        ident[:])
                    nc.vector.tensor_copy(convT[:, m, w * 128:(w + 1) * 128],
                                          trp[:])

            raw4 = spool.tile([128, 3, NCH, 6], fp32, tag="l4_bs")

            def conv_stats(j):
                # bn_stats on the 512-node chunk [j*NCOL, (j+1)*NCOL) as soon
                # as its 4 windows are finalized (overlaps the conv loop)
                for m in range(3):
                    nc.vector.bn_stats(raw4[:, m, j:j + 1, :],
                                       convT[:, m, j * NCOL:(j + 1) * NCOL])

            t = 0
            nq = 0
            while t < T_total:
                cn = min(GCHUNK, T_total - t)
                gt = gpool.tile([128, GCHUNK * D], bf16, tag="g")
                nc.gpsimd.dma_gather(
                    out_ap=gt[:, :cn * D].rearrange("p (t f) -> p t f", f=D),
                    in_ap=y_full[:],
                    idxs_ap=idx_t[:, t * 8:(t + cn) * 8],
                    num_idxs=cn * 128, num_idxs_reg=cn * 128, elem_size=D)
                gv = gt[:, :cn * D].rearrange("p (t f) -> p t f", f=D)
                for tl in range(cn):
                    tg = t + tl
                    w = int(win_of[tg])
                    if tg == tstart[w]:
                        ps_win[w] = cvps.tile([128, D], fp32, name="ps_c",
                                              tag="cv")
                    sel = selp.tile([128, 128], bf16, tag="sel")
                    nc.vector.tensor_scalar(sel[:], iota[:], dst_t[:, tg:tg + 1],
                                            None, ALU.is_equal)
                    last = (tg == tstart[w + 1] - 1)
                    nc.tensor.matmul(ps_win[w][:], lhsT=sel[:],
                                     rhs=gv[:, tl, :],
                                     start=(tg == tstart[w]), stop=last)
                    if last:
                        finalize_window(w)
                        if w % 4 == 3:
                            conv_stats(w // 4)
                t += cn

            st4 = spool.tile([128, 3, 2], fp32, tag="st4")
            _combine_bn_raw(raw4, st4, 3)
            red4 = ag_exchange(st4[:], 6, "ar4")
            gp4, bp4 = bn_coeffs(red4, 3, N // 256, "g4", "be4", "bn4")
            bn_apply(convT, gp4, bp4, 3, NPC)
            tap("convT", convT[:])
            if stage <= 4:
                finish_early(convT[0:C, 0, 0:GPC])
                continue

            # ---------------- post MLPs
            p1, st5 = mlp_layer(convT, W["wp1"], vcol("bp1"), "l5")
            red5 = ag_exchange(st5[:], 6, "ar5")
            gp5, bp5 = bn_coeffs(red5, 3, N // 256, "g5", "be5", "bn5")
            bn_apply(p1, gp5, bp5, 3, NPC)

            # post2: row-major bf16 to DRAM (pre-BN); stats collected
            p2_dram = dram.tile([NPC, D], bf16, tag="p2")
            raw6 = spool.tile([128, 3, NCH, 6], fp32, tag="p2_bs")
            for j in range(NCH):
                sl = slice(j * NCOL, (j + 1) * NCOL)
                p2c = spool.tile([128, 3, NCOL], bf16, tag="p2c")
                for m in range(3):
                    ps = mmps.tile([128, NCOL], fp32, tag="mm")
                    for k in range(3):
                        nc.tensor.matmul(
                            ps[:], lhsT=W["wp2"][:, k, m * 128:(m + 1) * 128],
                            rhs=p1[:, k, sl], start=(k == 0), stop=(k == 2))
                    nc.scalar.activation(p2c[:, m, :], ps[:], AF.Relu,
                                         bias=vcol("bp2")[:, m:m + 1])
                    nc.vector.bn_stats(raw6[:, m, j:j + 1, :], p2c[:, m, :])
                for nb in range(NCOL // 128):
                    rmw = spool.tile([128, D], bf16, tag="rmw")
                    for m in range(3):
                        trp = trps.tile([128, 128], bf16, tag="tr")
                        nc.tensor.transpose(
                            trp[:], p2c[:, m, nb * 128:(nb + 1) * 128], ident[:])
                        nc.vector.tensor_copy(rmw[:, m * 128:(m + 1) * 128], trp[:])
                    nc.sync.dma_start(
                        p2_dram[j * NCOL + nb * 128:j * NCOL + (nb + 1) * 128, :],
                        rmw[:])
            # combine raw6 -> st6 [128, 3, 2] (sum/256, sumsq/256)
            st6 = spool.tile([128, 3, 2], fp32, tag="st6")
            _combine_bn_raw(raw6, st6, 3)
            red6 = ag_exchange(st6[:], 6, "ar6")
            gp6, bp6 = bn_coeffs(red6, 3, N // 256, "g6", "be6", "bn6")
            if stage <= 5:
                finish_early(red6[0:C, 0:6], width=6)
                continue

            # ---------------- masked-node gather -> flT [128, 3, 16] fp32 (BN6'd)
            gth = spool.tile([128, D], bf16, tag="gth")
            nc.gpsimd.dma_gather(
                out_ap=gth[:].rearrange("p (t f) -> p t f", f=D),
                in_ap=p2_dram[:], idxs_ap=gidx_t[:],
                num_idxs=16, num_idxs_reg=16, elem_size=D)
            flT = spool.tile([128, 3, 16], bf16, tag="flT")
            for m in range(3):
                trp_full = trps.tile([128, 128], bf16, tag="tr")
                trp = trp_full[:, 0:16]
                nc.tensor.matmul(trp, lhsT=gth[0:16, m * 128:(m + 1) * 128],
                                 rhs=ident[0:16, 0:16], is_transpose=True)
                nc.vector.tensor_scalar(flT[:, m, :], trp,
                                        gp6[:, m:m + 1], bp6[:, m:m + 1],
                                        ALU.mult, ALU.add)
            tap("flT", flT[:])

            # ---------------- tail: outc, H_sent BN, att, out
            outcT = spool.tile([128, 6, GPC], fp32, tag="outcT")
            for m in range(6):
                ps_o = typs.tile([128, GPC], fp32, tag="tiny")
                for k in range(6):
                    kc, kj = k % 3, k // 3
                    nc.tensor.matmul(
                        ps_o[:], lhsT=wcat[:, k, m * 128:(m + 1) * 128],
                        rhs=flT[:, kc, kj::2], start=(k == 0), stop=(k == 5))
                nc.scalar.activation(outcT[:, m, :], ps_o[:], AF.Relu,
                                     bias=vcol("bcat")[:, m:m + 1])
            stt = spool.tile([128, 24], fp32, tag="stt")
            for m in range(6):
                nc.vector.reduce_sum(stt[:, 2 * m:2 * m + 1], outcT[:, m, :], axis=AX.X)
                sq = spool.tile([128, GPC], fp32, tag="ttsq")
                nc.scalar.square(sq[:], outcT[:, m, :])
                nc.vector.reduce_sum(stt[:, 2 * m + 1:2 * m + 2], sq[:], axis=AX.X)
                nc.vector.reduce_sum(stt[:, 12 + 2 * m:13 + 2 * m], HsT[:, m, :], axis=AX.X)
                nc.scalar.square(sq[:], HsT[:, m, :])
                nc.vector.reduce_sum(stt[:, 13 + 2 * m:14 + 2 * m], sq[:], axis=AX.X)
            redt = ag_exchange(stt[:], 24, "art")
            gpc_, bpc_ = bn_coeffs(redt[:, 0:12], 6, B, "gc0", "bc0", "bnc")
            gph, bph = bn_coeffs(redt[:, 12:24], 6, B, "gc1", "bc1", "bnh")
            attT = spool.tile([128, 6, GPC], fp32, tag="attT")
            for m in range(6):
                nc.vector.tensor_scalar(attT[:, m, :], HsT[:, m, :],
                                        gph[:, m:m + 1], bph[:, m:m + 1],
                                        ALU.mult, ALU.add)
                nc.vector.tensor_scalar(outcT[:, m, :], outcT[:, m, :],
                                        gpc_[:, m:m + 1], bpc_[:, m:m + 1],
                                        ALU.mult, ALU.add)
                nc.vector.tensor_add(attT[:, m, :], attT[:, m, :], outcT[:, m, :])
            ps_ft = typs.tile([128, GPC], fp32, tag="tiny")
            ps_f = ps_ft[0:C, :]
            for k in range(6):
                kc, kj = k % 3, k // 3
                nc.tensor.matmul(ps_f, lhsT=wout[:, k, :], rhs=attT[:, k, :],
                                 start=(k == 0), stop=False)
            nc.tensor.matmul(ps_f, lhsT=brow[0:1, D:D + C], rhs=ones8[:],
                             start=False, stop=True)
            fin = spool.tile([C, GPC], fp32, tag="fin")
            nc.vector.tensor_copy(fin[:], ps_f)
            nc.sync.dma_start(outT[:], fin[:])

    nc.compile()
    return nc, tap_outs


# ---------------------------------------------------------------- entry
_CACHE = {}


def _get_compiled(meta):
    key = meta
    if key not in _CACHE:
        nc, _ = build(meta)
        split_waits(nc)
        _CACHE[key] = nc
    return _CACHE[key]


def kernel(**inputs):
    in_maps, meta = preprocess(inputs)
    nc = _get_compiled(meta)
    from concourse import bass2jax
    results = bass2jax.run_bass_via_pjrt(nc, in_maps, n_cores=NCORES)
    out = np.concatenate([results[c]["outT"].T for c in range(NCORES)], axis=0)
    return out.astype(np.float32)


# revision 30
# speedup vs baseline: 1.0238x; 1.0238x over previous
"""Trainium2 Bass kernel for nn_BaselineModelWithGNN (8-core SPMD).

Self-contained: hardcodes shapes/sharding; builds, compiles and runs the Bass
program on 8 NeuronCores via the axon PJRT path.

Key observation: the reference applies each of the 3 GCN convs to the same
input x and overwrites `out`, so only conv i=2 (w_conv[2], b_conv[2],
bng[4]) affects the result — one conv is computed.

Sharding: nodes (and their incident edges, dst-sharded) are partitioned
contiguously across the 8 cores (4096 nodes / 8 graphs per core); the
PLM/pooling branch is data-parallel over batch. BatchNorm statistics are
exchanged with a small AllGather + local combine; the 384-wide node
features y are AllGathered (fp16) for the edge gather.

Perf notes (CoreSim cost model):
- selector one-hots use fp16 iota/dstloc so TensorScalarPtr hits the 4x DVE
  mode (all-2-byte operands).
- BN statistics via the native bn_stats instruction (one pass, sum+var).
- dma_gather chunks span dst-window boundaries (16 tiles = 2048 descriptors
  per call; dynamic_dma_scratch_size=65536 gives a 4096-descriptor SWDGE
  ring) to amortize the ~1us fixed SWDGE overhead per gather.
- stat exchange = AllGather (15.6us) + 7 local adds, cheaper than AllReduce
  (28.2us) under the collective cost model.
"""
import sys
sys.path.insert(0, "/opt/trn_rl_repo")
from contextlib import ExitStack

import numpy as np
import ml_dtypes

import bass_rust as _br
import concourse.bacc as bacc
import concourse.bass as bass
import concourse.tile as tile
from concourse import mybir
from concourse._compat import cdiv

fp32 = mybir.dt.float32
bf16 = mybir.dt.float16  # "bf16" name kept; fp16 for 8x less quant noise
fp8 = mybir.dt.float8e4
i16 = mybir.dt.int16
AF = mybir.ActivationFunctionType
ALU = mybir.AluOpType
AX = mybir.AxisListType

NCORES = 8
B, S, E = 64, 512, 768
D = 384
NG = 512
N = B * NG              # 32768
NEDGE = 1048576
C = 3
NPC = N // NCORES       # 4096 nodes per core
GPC = B // NCORES       # 8 graphs per core
NW = NPC // 128         # 32 dst windows per core
NCOL = 512
NCH = NPC // NCOL       # 8 column chunks
EPS = 1e-5
GCHUNK = 8              # gather chunk: tiles (of 128 edges) per dma_gather;
                        # 1024 descs = the SWDGE ring; bigger wedges real HW
DPAD = 512              # y-table row bytes (fp8, padded): >=512B avoids the
                        # 2x DMA latency multiplier for sub-512B descriptors


# ---------------------------------------------------------------- BIR patch
def split_waits(nc):
    """walrus here supports ONE sync-wait per instruction; split extras onto
    NoOps inserted just before, on the same engine."""
    counter = 0
    for f in nc.m.functions:
        for bb in f.blocks:
            newlist, changed = [], False
            for inst in bb.instructions:
                si = inst.sync_info
                if si is not None and len(si.on_wait) > 1:
                    waits = list(si.on_wait)
                    for w in waits[:-1]:
                        counter += 1
                        nop = mybir.InstNoOp(name=f"I-WSPLIT-{counter}", ins=[], outs=[])
                        nop.engine = inst.engine
                        nop.sync_info = _br.SyncInfo(on_wait=[w], on_update=[])
                        newlist.append(nop)
                    inst.sync_info = _br.SyncInfo(
                        on_wait=[waits[-1]], on_update=list(si.on_update))
                    changed = True
                newlist.append(inst)
            if changed:
                bb.instructions = newlist


# ---------------------------------------------------------------- host prep
def _col3(v):
    """[384] -> [128, 3] column layout (feature f = c*128+p)."""
    return np.ascontiguousarray(v.reshape(3, 128).T).astype(np.float32)


def _col6(v):
    return np.ascontiguousarray(v.reshape(6, 128).T).astype(np.float32)


def _wchunks(w, kc, m):
    """[K, M] -> [128, kc, M] (k-chunk on partitions)."""
    K, M = w.shape
    assert K == kc * 128
    return np.ascontiguousarray(w.reshape(kc, 128, M).transpose(1, 0, 2))


def _wrap_idx(idx):
    """int16 idx array (len % 128 == 0) -> [128, len/16] dma_gather layout."""
    blk = idx.reshape(-1, 16).T  # [16, len/16]
    return np.ascontiguousarray(np.tile(blk, (8, 1)))


def preprocess(inputs):
    ei = np.asarray(inputs["edge_index"]).astype(np.int64)
    loop = np.arange(N, dtype=np.int64)
    src_all = np.concatenate([ei[0], loop])
    dst_all = np.concatenate([ei[1], loop])

    deg = np.bincount(dst_all, minlength=N).astype(np.float32)  # CSR row lengths

    # src-sharded conv: core c owns edges whose src is in its node range and
    # accumulates partial sums over ALL 256 global dst windows; a bf16
    # ReduceScatter then sums and distributes rows back to their dst owners.
    score = src_all // NPC
    gwin = dst_all >> 7
    order = np.lexsort((src_all, gwin, score))
    src_s = src_all[order]
    dst_s = dst_all[order]
    score_s = score[order]
    gwin_s = gwin[order]
    cstart = np.searchsorted(score_s, np.arange(NCORES + 1))

    NWG = N // 128                                        # 256 global windows
    cnts = np.zeros((NCORES, NWG), np.int64)
    for c in range(NCORES):
        cnts[c] = np.bincount(gwin_s[cstart[c]:cstart[c + 1]], minlength=NWG)
    tpw = np.maximum(np.ceil(cnts / 128).max(axis=0), 1).astype(np.int64)
    T_total = int(tpw.sum())
    tstart = np.zeros(NWG + 1, np.int64)
    np.cumsum(tpw, out=tstart[1:])

    # masked node indices (2 per graph, ascending)
    mask = np.asarray(inputs["graph_masking"])
    sel = np.argsort(-mask, axis=1, kind="stable")[:, :2]  # top_k: ones, asc idx
    sel = np.sort(sel, axis=1)

    xT = np.zeros((D, N), np.float32)
    xT[:300] = np.asarray(inputs["x_nodes"]).T
    xT = xT.astype(np.float16)

    w1p = np.zeros((D, D), np.float32)
    w1p[:300] = np.asarray(inputs["w_pre1"])

    lastf = np.asarray(inputs["last_h"]).astype(np.float16)
    firstf = np.asarray(inputs["first_h"]).astype(np.float16)

    bng_g, bng_b = np.asarray(inputs["bng_g"]), np.asarray(inputs["bng_b"])
    bn_g, bn_b = np.asarray(inputs["bn_g"]), np.asarray(inputs["bn_b"])
    # vec columns [128, 72]: order documented here, mirrored on device
    cols = [
        _col3(np.asarray(inputs["b_pre1"])), _col3(np.asarray(inputs["b_pre2"])),
        _col3(np.asarray(inputs["b_post1"])), _col3(np.asarray(inputs["b_post2"])),
        _col6(np.asarray(inputs["b_cat"])),
        _col3(bng_g[0]), _col3(bng_b[0]), _col3(bng_g[1]), _col3(bng_b[1]),
        _col3(bng_g[4]), _col3(bng_b[4]), _col3(bng_g[5]), _col3(bng_b[5]),
        _col3(bng_g[6]), _col3(bng_b[6]),
        _col6(bn_g[0]), _col6(bn_b[0]), _col6(bn_g[1]), _col6(bn_b[1]),
    ]
    vecs = np.concatenate(cols, axis=1)  # [128, 3*4+6+3*10+6*4] = [128, 72]
    brow = np.zeros((1, 512), np.float32)
    brow[0, :D] = np.asarray(inputs["b_conv"])[2]
    brow[0, D:D + C] = np.asarray(inputs["b_out"])

    w_bf = {
        "w1": _wchunks(w1p, 3, D).astype(np.float16),
        "w2": _wchunks(np.asarray(inputs["w_pre2"]), 3, D).astype(np.float16),
        "wc": _wchunks(np.asarray(inputs["w_conv"])[2], 3, D).astype(np.float16),
        "wp1": _wchunks(np.asarray(inputs["w_post1"]), 3, D).astype(np.float16),
        "wp2": _wchunks(np.asarray(inputs["w_post2"]), 3, D).astype(np.float16),
    }
    wcat = _wchunks(np.asarray(inputs["w_cat"]), 6, E).astype(np.float16)
    wout = _wchunks(np.asarray(inputs["w_out"]), 6, C).astype(np.float32)

    in_maps = []
    for c in range(NCORES):
        n0 = c * NPC
        src_pad = np.zeros(T_total * 128, np.int64)
        dstloc = np.full(T_total * 128, -1.0, np.float32)
        base = cstart[c]
        wofs = np.zeros(NWG + 1, np.int64)
        np.cumsum(cnts[c], out=wofs[1:])
        for gw in range(NWG):
            a, b_ = base + wofs[gw], base + wofs[gw + 1]
            k = b_ - a
            pos = tstart[gw] * 128
            src_pad[pos:pos + k] = src_s[a:b_] - n0          # local src idx
            dstloc[pos:pos + k] = (dst_s[a:b_] - gw * 128).astype(np.float32)
        idx_w = _wrap_idx(src_pad.astype(np.int16))            # [128, T*8]
        dst_t = np.ascontiguousarray(dstloc.reshape(T_total, 128).T)  # [128, T]

        deg_nm = np.ascontiguousarray(
            deg[n0:n0 + NPC].reshape(NW, 128).T)               # [128, 32]

        gidx = (sel[c * GPC:(c + 1) * GPC] +
                np.arange(c * GPC, (c + 1) * GPC)[:, None] * NG - n0)  # local
        gidx = gidx.reshape(-1).astype(np.int16)               # [16]
        gidx_w = np.zeros((128, 1), np.int16)
        gidx_w[:16, 0] = gidx
        gidx_w = np.tile(gidx_w[:16], (8, 1))

        m = {
            "lasth": np.ascontiguousarray(
                lastf[c * GPC:(c + 1) * GPC].reshape(GPC * S, E)),
            "firsth": np.ascontiguousarray(
                firstf[c * GPC:(c + 1) * GPC].reshape(GPC * S, E)),
            "xT": np.ascontiguousarray(
                xT.reshape(3, 128, N)[:, :, n0:n0 + NPC].transpose(1, 0, 2)
            ).reshape(128, 3 * NPC),
            "eidx": idx_w, "dstloc": dst_t, "deg": deg_nm,
            "vecs": vecs, "brow": brow, "gidx": gidx_w,
            "w1": w_bf["w1"].reshape(128, 3 * D),
            "w2": w_bf["w2"].reshape(128, 3 * D),
            "wc": w_bf["wc"].reshape(128, 3 * D),
            "wp1": w_bf["wp1"].reshape(128, 3 * D),
            "wp2": w_bf["wp2"].reshape(128, 3 * D),
            "wcat": wcat.reshape(128, 6 * E),
            "wout": wout.reshape(128, 6 * C),
        }
        in_maps.append(m)
    meta = (tuple(int(t) for t in tpw),)
    return in_maps, meta


# ---------------------------------------------------------------- device
def build(meta, rep=1, taps=(), stage=99):
    tpw = meta[0]
    T_total = sum(tpw)
    tstart = [0]
    for t in tpw:
        tstart.append(tstart[-1] + t)
    win_of = np.zeros(T_total, np.int64)
    for w in range(len(tpw)):
        win_of[tstart[w]:tstart[w + 1]] = w

    nc = bacc.Bacc("TRN2")
    I = {}
    I["lasth"] = nc.dram_tensor("lasth", [GPC * S, E], bf16, kind="ExternalInput")
    I["firsth"] = nc.dram_tensor("firsth", [GPC * S, E], bf16, kind="ExternalInput")
    I["xT"] = nc.dram_tensor("xT", [128, 3 * NPC], bf16, kind="ExternalInput")
    I["eidx"] = nc.dram_tensor("eidx", [128, T_total * 8], i16, kind="ExternalInput")
    I["dstloc"] = nc.dram_tensor("dstloc", [128, T_total], fp32, kind="ExternalInput")
    I["deg"] = nc.dram_tensor("deg", [128, NW], fp32, kind="ExternalInput")
    I["vecs"] = nc.dram_tensor("vecs", [128, 72], fp32, kind="ExternalInput")
    I["brow"] = nc.dram_tensor("brow", [1, 512], fp32, kind="ExternalInput")
    I["gidx"] = nc.dram_tensor("gidx", [128, 1], i16, kind="ExternalInput")
    for w in ("w1", "w2", "wc", "wp1", "wp2"):
        I[w] = nc.dram_tensor(w, [128, 3 * D], bf16, kind="ExternalInput")
    I["wcat"] = nc.dram_tensor("wcat", [128, 6 * E], bf16, kind="ExternalInput")
    I["wout"] = nc.dram_tensor("wout", [128, 6 * C], fp32, kind="ExternalInput")
    outT = nc.dram_tensor("outT", [C, GPC], fp32, kind="ExternalOutput")
    tap_outs = {}

    grp = [list(range(NCORES))]

    with tile.TileContext(nc) as tc, ExitStack() as ctx:
        const = ctx.enter_context(tc.tile_pool(name="const", bufs=1))
        big = ctx.enter_context(tc.tile_pool(name="big", bufs=2))
        gpool = ctx.enter_context(tc.tile_pool(name="gath", bufs=2))
        spool = ctx.enter_context(tc.tile_pool(name="small", bufs=2))
        selp = ctx.enter_context(tc.tile_pool(name="sel", bufs=8))
        hspool = ctx.enter_context(tc.tile_pool(name="hs", bufs=8))
        cinp = ctx.enter_context(tc.tile_pool(name="cin", bufs=1))
        mmps = ctx.enter_context(tc.tile_pool(name="mmps", bufs=2, space="PSUM"))
        cvps = ctx.enter_context(tc.tile_pool(name="cvps", bufs=2, space="PSUM"))
        trps = ctx.enter_context(tc.tile_pool(name="trps", bufs=2, space="PSUM"))
        typs = ctx.enter_context(tc.tile_pool(name="typs", bufs=2, space="PSUM"))
        dram = ctx.enter_context(tc.tile_pool(name="dram", bufs=1, space="DRAM"))

        # ---------------- constants
        iota = const.tile([128, 128], bf16)
        nc.gpsimd.iota(iota[:], pattern=[[1, 128]], base=0, channel_multiplier=0,
                       allow_small_or_imprecise_dtypes=True)
        pidx = const.tile([128, 1], fp32)  # partition index column
        nc.gpsimd.iota(pidx[:], pattern=[[0, 1]], base=0, channel_multiplier=1,
                       allow_small_or_imprecise_dtypes=True)
        ident = const.tile([128, 128], bf16)
        nc.vector.tensor_scalar(ident[:], iota[:], pidx[:], None, ALU.is_equal)

        idx_t = const.tile([128, T_total * 8], i16)
        nc.sync.dma_start(idx_t[:], I["eidx"][:])
        dst_t = const.tile([128, T_total], fp32)
        nc.sync.dma_start(dst_t[:], I["dstloc"][:])
        deg_t = const.tile([128, NW], fp32)
        nc.sync.dma_start(deg_t[:], I["deg"][:])
        vecs = const.tile([128, 72], fp32)
        nc.sync.dma_start(vecs[:], I["vecs"][:])
        brow = const.tile([1, 512], fp32)
        nc.sync.dma_start(brow[:], I["brow"][:])
        gidx_t = const.tile([128, 1], i16)
        nc.sync.dma_start(gidx_t[:], I["gidx"][:])
        W = {}
        for w in ("w1", "w2", "wc", "wp1", "wp2"):
            W[w] = const.tile([128, 3, D], bf16, name=f"W_{w}", tag=f"W_{w}")
            nc.sync.dma_start(W[w][:], I[w][:].rearrange("p (k m) -> p k m", k=3))
        wcat = const.tile([128, 6, E], bf16)
        nc.sync.dma_start(wcat[:], I["wcat"][:].rearrange("p (k m) -> p k m", k=6))
        wout = const.tile([128, 6, C], fp32)
        nc.sync.dma_start(wout[:], I["wout"][:].rearrange("p (k m) -> p k m", k=6))
        bconv_bc = const.tile([128, D], fp32)
        nc.gpsimd.partition_broadcast(bconv_bc[:], brow[0:1, 0:D])
        ones8 = const.tile([1, GPC], fp32)
        nc.vector.memset(ones8[:], 1.0)
        onescol = const.tile([128, 1], bf16)
        nc.vector.memset(onescol[:], 1.0)

        # vec column offsets
        VO = {}
        off = 0
        for name, w_ in [("b1", 3), ("b2", 3), ("bp1", 3), ("bp2", 3), ("bcat", 6),
                         ("g0", 3), ("be0", 3), ("g1", 3), ("be1", 3),
                         ("g4", 3), ("be4", 3), ("g5", 3), ("be5", 3),
                         ("g6", 3), ("be6", 3),
                         ("gc0", 6), ("bc0", 6), ("gc1", 6), ("bc1", 6)]:
            VO[name] = (off, w_)
            off += w_
        def vcol(name):
            o, w_ = VO[name]
            return vecs[:, o:o + w_]

        # deg^-1/2
        dinv = const.tile([128, NW], fp32)
        nc.scalar.sqrt(dinv[:], deg_t[:])
        nc.vector.reciprocal(dinv[:], dinv[:])

        def tap(name, ap):
            if name not in taps:
                return
            t_ = nc.dram_tensor(f"tap_{name}", list(ap.shape), ap.dtype,
                                kind="ExternalOutput")
            tap_outs[name] = t_
            nc.sync.dma_start(t_[:], ap)

        def finish_early(src_ap, width=GPC):
            fin0 = spool.tile([C, GPC], fp32, name="fin0", tag="fin")
            nc.vector.memset(fin0[:], 0.0)
            nc.vector.tensor_scalar(fin0[:, 0:width], src_ap, 1.0, None, ALU.mult)
            nc.sync.dma_start(outT[:], fin0[:])

        for _rep in range(rep):
            # ---------------- helpers
            def _combine_bn_raw(raw, st, m_chunks):
                """raw [128, m, NCH, 6] bn_stats outputs -> st [128, m, 2]
                holding (sum/256, sumsq/256) per feature."""
                Ev = raw[:, :, :, 1]
                Vv = raw[:, :, :, 2]
                Ov = raw[:, :, :, 4]
                Wv = raw[:, :, :, 5]
                e2 = spool.tile([128, m_chunks, NCH], fp32, name="e2", tag="cb_e2")
                o2 = spool.tile([128, m_chunks, NCH], fp32, name="o2", tag="cb_o2")
                vv = spool.tile([128, m_chunks, NCH], fp32, name="vv", tag="cb_vv")
                ss = spool.tile([128, m_chunks, NCH], fp32, name="ss", tag="cb_ss")
                nc.vector.tensor_mul(e2[:], Ev, Ev)
                nc.vector.tensor_mul(o2[:], Ov, Ov)
                nc.vector.tensor_add(e2[:], e2[:], o2[:])
                nc.vector.tensor_tensor(vv[:], Vv, Wv, ALU.add)
                nc.vector.scalar_tensor_tensor(
                    vv[:], vv[:], 1.0 / 256, e2[:], op0=ALU.mult, op1=ALU.add)
                nc.vector.tensor_tensor(ss[:], Ev, Ov, ALU.add)
                nc.vector.reduce_sum(st[:, :, 0:1], ss[:], axis=AX.X)
                nc.vector.reduce_sum(st[:, :, 1:2], vv[:], axis=AX.X)

            def bn_stats_pack(src_view, m_chunks, tag):
                """src_view(m) -> [128, NCH, NCOL] bf16 view; returns
                [128, m_chunks*2] (sum/256, sumsq/256) stat tile."""
                raw = spool.tile([128, m_chunks, NCH, 6], fp32, tag=tag + "_bs")
                for m in range(m_chunks):
                    sv = src_view(m)
                    for j in range(NCH):
                        nc.vector.bn_stats(raw[:, m, j:j + 1, :], sv[:, j, :])
                st = spool.tile([128, m_chunks, 2], fp32, tag=tag + "_st")
                _combine_bn_raw(raw, st, m_chunks)
                return st

            def ag_exchange(st_ap, width, tag):
                """AllGather the [128, width] stat tile; sum over ranks."""
                cin = dram.tile([128, width], fp32, tag=tag + "_ci")
                cout = dram.tile([NCORES * 128, width], fp32, tag=tag + "_co")
                nc.sync.dma_start(cin[:], st_ap)
                nc.gpsimd.collective_compute(
                    "AllGather", ALU.bypass, replica_groups=grp,
                    ins=[cin[:]], outs=[cout[:]])
                allsb = spool.tile([128, NCORES, width], fp32, tag=tag + "_as")
                nc.sync.dma_start(
                    allsb[:], cout[:].rearrange("(r p) w -> p r w", p=128))
                red = spool.tile([128, width], fp32, tag=tag + "_rd")
                nc.vector.tensor_tensor(red[:], allsb[:, 0], allsb[:, 1], ALU.add)
                for r in range(2, NCORES):
                    nc.vector.tensor_tensor(red[:], red[:], allsb[:, r], ALU.add)
                return red

            def bn_coeffs(red, m_chunks, count, gname, bname, tag):
                """red [128, 2*m]: per-feature sum/sumsq -> gp, bp [128, m]"""
                gp = spool.tile([128, m_chunks], fp32, tag=tag + "_gp")
                bp = spool.tile([128, m_chunks], fp32, tag=tag + "_bp")
                mu = spool.tile([128, m_chunks], fp32, tag=tag + "_mu")
                var = spool.tile([128, m_chunks], fp32, tag=tag + "_va")
                inv_n = 1.0 / count
                sview = red.rearrange("p (m two) -> p m two", two=2)
                nc.vector.tensor_scalar(mu[:], sview[:, :, 0], inv_n, None, ALU.mult)
                nc.vector.tensor_scalar(var[:], sview[:, :, 1], inv_n, None, ALU.mult)
                musq = spool.tile([128, m_chunks], fp32, tag=tag + "_ms")
                nc.vector.tensor_mul(musq[:], mu[:], mu[:])
                nc.vector.tensor_tensor(var[:], var[:], musq[:], ALU.subtract)
                nc.vector.tensor_scalar(var[:], var[:], EPS, None, ALU.add)
                nc.scalar.sqrt(var[:], var[:])
                nc.vector.reciprocal(var[:], var[:])          # 1/sigma
                nc.vector.tensor_mul(gp[:], vcol(gname), var[:])
                nc.vector.tensor_mul(bp[:], gp[:], mu[:])
                nc.vector.tensor_tensor(bp[:], vcol(bname), bp[:], ALU.subtract)
                return gp, bp

            def bn_apply(x_t, gp, bp, m_chunks, width):
                for m in range(m_chunks):
                    nc.vector.tensor_scalar(
                        x_t[:, m, :width], x_t[:, m, :width],
                        gp[:, m:m + 1], bp[:, m:m + 1], ALU.mult, ALU.add)

            def mlp_layer(x_t, w_t, bias_col, tag):
                """x_t [128,3,NPC] bf16 -> relu(x@W + b) bf16 [128,3,NPC] + stats"""
                out = big.tile([128, 3, NPC], bf16, tag="big")
                for m in range(3):
                    for j in range(NCH):
                        sl = slice(j * NCOL, (j + 1) * NCOL)
                        ps = mmps.tile([128, NCOL], fp32, tag="mm")
                        for k in range(3):
                            nc.tensor.matmul(
                                ps[:], lhsT=w_t[:, k, m * 128:(m + 1) * 128],
                                rhs=x_t[:, k, sl], start=(k == 0), stop=(k == 2))
                        nc.scalar.activation(out[:, m, sl], ps[:], AF.Relu,
                                             bias=bias_col[:, m:m + 1])
                st = bn_stats_pack(
                    lambda m: out[:, m, :].rearrange("p (j n) -> p j n", n=NCOL),
                    3, tag)
                return out, st

            # ---------------- pre-MLPs
            xT_t = big.tile([128, 3, NPC], bf16, tag="big")
            nc.sync.dma_start(xT_t[:], I["xT"][:].rearrange("p (k n) -> p k n", k=3))

            x1, st1 = mlp_layer(xT_t, W["w1"], vcol("b1"), "l1")
            red1 = ag_exchange(st1[:], 6, "ar1")
            gp1, bp1 = bn_coeffs(red1, 3, N // 256, "g0", "be0", "bn1")
            bn_apply(x1, gp1, bp1, 3, NPC)
            tap("x1", x1[:])

            x2, st2 = mlp_layer(x1, W["w2"], vcol("b2"), "l2")
            red2 = ag_exchange(st2[:], 6, "ar2")
            gp2, bp2 = bn_coeffs(red2, 3, N // 256, "g1", "be1", "bn2")
            bn_apply(x2, gp2, bp2, 3, NPC)
            tap("x2", x2[:])
            if stage <= 2:
                finish_early(x2[0:C, 0, 0:GPC])
                continue

            # ---------------- z = x2 @ wc ; y = z * dinv (node-major, fp8
            # rows padded to DPAD bytes so each gather descriptor is >=512B)
            y_slice = dram.tile([NPC, D], bf16, tag="y_slice")
            z_sb = big.tile([128, 3, NPC], bf16, tag="big")
            for m in range(3):
                for j in range(NCH):
                    sl = slice(j * NCOL, (j + 1) * NCOL)
                    ps = mmps.tile([128, NCOL], fp32, tag="mm")
                    for k in range(3):
                        nc.tensor.matmul(
                            ps[:], lhsT=W["wc"][:, k, m * 128:(m + 1) * 128],
                            rhs=x2[:, k, sl], start=(k == 0), stop=(k == 2))
                    nc.scalar.activation(z_sb[:, m, sl], ps[:], AF.Copy)
            for w in range(NW):
                ywin = spool.tile([128, D], bf16, tag="ywin")
                for m in range(3):
                    trp = trps.tile([128, 128], bf16, tag="tr")
                    nc.tensor.transpose(
                        trp[:], z_sb[:, m, w * 128:(w + 1) * 128], ident[:])
                    nc.vector.tensor_scalar(
                        ywin[:, m * 128:(m + 1) * 128], trp[:],
                        dinv[:, w:w + 1], None, ALU.mult)
                nc.sync.dma_start(y_slice[w * 128:(w + 1) * 128, :], ywin[:])

            # ---------------- conv: per-core partial sums over ALL 256 global
            # dst windows (gathering from the LOCAL y_slice only), written
            # bf16 to pagg, then summed+sharded with one ReduceScatter.
            pagg = dram.tile([N, D], bf16, tag="pagg")
            ps_win = {}

            def partial_flush(gw):
                ps_c = ps_win.pop(gw)
                pb = spool.tile([128, D], bf16, name="pb", tag="pb")
                nc.scalar.activation(pb[:], ps_c[:], AF.Copy)
                nc.sync.dma_start(pagg[gw * 128:(gw + 1) * 128, :], pb[:])

            t = 0
            while t < T_total:
                cn = min(GCHUNK, T_total - t)
                gt = gpool.tile([128, GCHUNK * D], bf16, tag="g")
                nc.gpsimd.dma_gather(
                    out_ap=gt[:, :cn * D].rearrange("p (t f) -> p t f", f=D),
                    in_ap=y_slice[:],
                    idxs_ap=idx_t[:, t * 8:(t + cn) * 8],
                    num_idxs=cn * 128, num_idxs_reg=cn * 128, elem_size=D)
                gv = gt[:, :cn * D].rearrange("p (t f) -> p t f", f=D)
                for tl in range(cn):
                    tg = t + tl
                    w = int(win_of[tg])
                    if tg == tstart[w]:
                        ps_win[w] = cvps.tile([128, D], fp32, name="ps_c",
                                              tag="cv")
                    sel = selp.tile([128, 128], bf16, tag="sel")
                    nc.vector.tensor_scalar(sel[:], iota[:], dst_t[:, tg:tg + 1],
                                            None, ALU.is_equal)
                    last = (tg == tstart[w + 1] - 1)
                    nc.tensor.matmul(ps_win[w][:], lhsT=sel[:],
                                     rhs=gv[:, tl, :],
                                     start=(tg == tstart[w]), stop=last)
                    if last:
                        partial_flush(w)
                t += cn

            conv_in = dram.tile([NPC, D], bf16, tag="conv_in")
            nc.gpsimd.collective_compute(
                "ReduceScatter", ALU.add, replica_groups=grp,
                ins=[pagg[:]], outs=[conv_in[:]])

            # ---------------- sentence branch (overlaps the ReduceScatter)
            HsT = spool.tile([128, 6, GPC], fp32, tag="HsT")
            for b in range(GPC):
                ps_ht = typs.tile([128, GPC], fp32, tag="tiny")
                hts = []
                for hsrc in (I["lasth"], I["firsth"]):
                    for sc in range(4):
                        ht = hspool.tile([128, E], bf16, name="ht", tag="ht")
                        nc.sync.dma_start(
                            ht[:], hsrc[b * S + sc * 128:b * S + (sc + 1) * 128, :])
                        hts.append(ht)
                for m in range(6):
                    for i, ht in enumerate(hts):
                        nc.tensor.matmul(
                            ps_ht[:, m:m + 1],
                            lhsT=ht[:, m * 128:(m + 1) * 128],
                            rhs=onescol[:],
                            start=(i == 0), stop=(i == 7))
                nc.vector.tensor_scalar(
                    HsT[:, :, b], ps_ht[:, 0:6],
                    1.0 / (2 * S), None, ALU.mult)
            tap("hsT", HsT[:])
            if stage <= 3:
                finish_early(HsT[0:C, 0, 0:GPC])
                continue

            # ---------------- own-shard finalize: scale/bias/relu/transpose
            convT = big.tile([128, 3, NPC], bf16, tag="big")
            cin_sb = cinp.tile([128, NW, D], bf16, tag="cin")
            nc.sync.dma_start(
                cin_sb[:], conv_in[:].rearrange("(w p) f -> p w f", p=128))
            raw4 = spool.tile([128, 3, NCH, 6], fp32, tag="l4_bs")
            for w in range(NW):
                winf = spool.tile([128, D], fp32, tag="winf")
                nc.vector.scalar_tensor_tensor(
                    winf[:], cin_sb[:, w, :], dinv[:, w:w + 1], bconv_bc[:],
                    op0=ALU.mult, op1=ALU.add)
                winb = spool.tile([128, D], bf16, tag="winb")
                nc.scalar.activation(winb[:], winf[:], AF.Relu)
                for m in range(3):
                    trp = trps.tile([128, 128], bf16, tag="tr")
                    nc.tensor.transpose(trp[:], winb[:, m * 128:(m + 1) * 128],
                                        ident[:])
                    nc.vector.tensor_copy(convT[:, m, w * 128:(w + 1) * 128],
                                          trp[:])
                if w % 4 == 3:
                    j = w // 4
                    for m in range(3):
                        nc.vector.bn_stats(raw4[:, m, j:j + 1, :],
                                           convT[:, m, j * NCOL:(j + 1) * NCOL])

            st4 = spool.tile([128, 3, 2], fp32, tag="st4")
            _combine_bn_raw(raw4, st4, 3)
            red4 = ag_exchange(st4[:], 6, "ar4")
            gp4, bp4 = bn_coeffs(red4, 3, N // 256, "g4", "be4", "bn4")
            bn_apply(convT, gp4, bp4, 3, NPC)
            tap("convT", convT[:])
            if stage <= 4:
                finish_early(convT[0:C, 0, 0:GPC])
                continue

            # ---------------- post MLPs
            p1, st5 = mlp_layer(convT, W["wp1"], vcol("bp1"), "l5")
            red5 = ag_exchange(st5[:], 6, "ar5")
            gp5, bp5 = bn_coeffs(red5, 3, N // 256, "g5", "be5", "bn5")
            bn_apply(p1, gp5, bp5, 3, NPC)

            # post2: row-major bf16 to DRAM (pre-BN); stats collected
            p2_dram = dram.tile([NPC, D], bf16, tag="p2")
            raw6 = spool.tile([128, 3, NCH, 6], fp32, tag="p2_bs")
            for j in range(NCH):
                sl = slice(j * NCOL, (j + 1) * NCOL)
                p2c = spool.tile([128, 3, NCOL], bf16, tag="p2c")
                for m in range(3):
                    ps = mmps.tile([128, NCOL], fp32, tag="mm")
                    for k in range(3):
                        nc.tensor.matmul(
                            ps[:], lhsT=W["wp2"][:, k, m * 128:(m + 1) * 128],
                            rhs=p1[:, k, sl], start=(k == 0), stop=(k == 2))
                    nc.scalar.activation(p2c[:, m, :], ps[:], AF.Relu,
                                         bias=vcol("bp2")[:, m:m + 1])
                    nc.vector.bn_stats(raw6[:, m, j:j + 1, :], p2c[:, m, :])
                for nb in range(NCOL // 128):
                    rmw = spool.tile([128, D], bf16, tag="rmw")
                    for m in range(3):
                        trp = trps.tile([128, 128], bf16, tag="tr")
                        nc.tensor.transpose(
                            trp[:], p2c[:, m, nb * 128:(nb + 1) * 128], ident[:])
                        nc.vector.tensor_copy(rmw[:, m * 128:(m + 1) * 128], trp[:])
                    nc.sync.dma_start(
                        p2_dram[j * NCOL + nb * 128:j * NCOL + (nb + 1) * 128, :],
                        rmw[:])
            # combine raw6 -> st6 [128, 3, 2] (sum/256, sumsq/256)
            st6 = spool.tile([128, 3, 2], fp32, tag="st6")
            _combine_bn_raw(raw6, st6, 3)
            red6 = ag_exchange(st6[:], 6, "ar6")
            gp6, bp6 = bn_coeffs(red6, 3, N // 256, "g6", "be6", "bn6")
            if stage <= 5:
                finish_early(red6[0:C, 0:6], width=6)
                continue

            # ---------------- masked-node gather -> flT [128, 3, 16] fp32 (BN6'd)
            gth = spool.tile([128, D], bf16, tag="gth")
            nc.gpsimd.dma_gather(
                out_ap=gth[:].rearrange("p (t f) -> p t f", f=D),
                in_ap=p2_dram[:], idxs_ap=gidx_t[:],
                num_idxs=16, num_idxs_reg=16, elem_size=D)
            flT = spool.tile([128, 3, 16], bf16, tag="flT")
            for m in range(3):
                trp_full = trps.tile([128, 128], bf16, tag="tr")
                trp = trp_full[:, 0:16]
                nc.tensor.matmul(trp, lhsT=gth[0:16, m * 128:(m + 1) * 128],
                                 rhs=ident[0:16, 0:16], is_transpose=True)
                nc.vector.tensor_scalar(flT[:, m, :], trp,
                                        gp6[:, m:m + 1], bp6[:, m:m + 1],
                                        ALU.mult, ALU.add)
            tap("flT", flT[:])

            # ---------------- tail: outc, H_sent BN, att, out
            outcT = spool.tile([128, 6, GPC], fp32, tag="outcT")
            for m in range(6):
                ps_o = typs.tile([128, GPC], fp32, tag="tiny")
                for k in range(6):
                    kc, kj = k % 3, k // 3
                    nc.tensor.matmul(
                        ps_o[:], lhsT=wcat[:, k, m * 128:(m + 1) * 128],
                        rhs=flT[:, kc, kj::2], start=(k == 0), stop=(k == 5))
                nc.scalar.activation(outcT[:, m, :], ps_o[:], AF.Relu,
                                     bias=vcol("bcat")[:, m:m + 1])
            stt = spool.tile([128, 24], fp32, tag="stt")
            for m in range(6):
                nc.vector.reduce_sum(stt[:, 2 * m:2 * m + 1], outcT[:, m, :], axis=AX.X)
                sq = spool.tile([128, GPC], fp32, tag="ttsq")
                nc.scalar.square(sq[:], outcT[:, m, :])
                nc.vector.reduce_sum(stt[:, 2 * m + 1:2 * m + 2], sq[:], axis=AX.X)
                nc.vector.reduce_sum(stt[:, 12 + 2 * m:13 + 2 * m], HsT[:, m, :], axis=AX.X)
                nc.scalar.square(sq[:], HsT[:, m, :])
                nc.vector.reduce_sum(stt[:, 13 + 2 * m:14 + 2 * m], sq[:], axis=AX.X)
            redt = ag_exchange(stt[:], 24, "art")
            gpc_, bpc_ = bn_coeffs(redt[:, 0:12], 6, B, "gc0", "bc0", "bnc")
            gph, bph = bn_coeffs(redt[:, 12:24], 6, B, "gc1", "bc1", "bnh")
            attT = spool.tile([128, 6, GPC], fp32, tag="attT")
            for m in range(6):
                nc.vector.tensor_scalar(attT[:, m, :], HsT[:, m, :],
                                        gph[:, m:m + 1], bph[:, m:m + 1],
                                        ALU.mult, ALU.add)
                nc.vector.tensor_scalar(outcT[:, m, :], outcT[:, m, :],
                                        gpc_[:, m:m + 1], bpc_[:, m:m + 1],
                                        ALU.mult, ALU.add)
                nc.vector.tensor_add(attT[:, m, :], attT[:, m, :], outcT[:, m, :])
            ps_ft = typs.tile([128, GPC], fp32, tag="tiny")
            ps_f = ps_ft[0:C, :]
            for k in range(6):
                kc, kj = k % 3, k // 3
                nc.tensor.matmul(ps_f, lhsT=wout[:, k, :], rhs=attT[:, k, :],
                                 start=(k == 0), stop=False)
            nc.tensor.matmul(ps_f, lhsT=brow[0:1, D:D + C], rhs=ones8[:],
                             start=False, stop=True)
            fin = spool.tile([C, GPC], fp32, tag="fin")
            nc.vector.tensor_copy(fin[:], ps_f)
            nc.sync.dma_start(outT[:], fin[:])

    nc.compile()
    return nc, tap_outs


# ---------------------------------------------------------------- entry
_CACHE = {}


def _get_compiled(meta):
    key = meta
    if key not in _CACHE:
        nc, _ = build(meta)
        split_waits(nc)
        _CACHE[key] = nc
    return _CACHE[key]


def kernel(**inputs):
    in_maps, meta = preprocess(inputs)
    nc = _get_compiled(meta)
    from concourse import bass2jax
    results = bass2jax.run_bass_via_pjrt(nc, in_maps, n_cores=NCORES)
    out = np.concatenate([results[c]["outT"].T for c in range(NCORES)], axis=0)
    return out.astype(np.float32)


# revision 38
# speedup vs baseline: 1.1362x; 1.1097x over previous
"""Trainium2 Bass kernel for nn_BaselineModelWithGNN (8-core SPMD).

Self-contained: hardcodes shapes/sharding; builds, compiles and runs the Bass
program on 8 NeuronCores via the axon PJRT path.

Key observation: the reference applies each of the 3 GCN convs to the same
input x and overwrites `out`, so only conv i=2 (w_conv[2], b_conv[2],
bng[4]) affects the result — one conv is computed.

Sharding: nodes (and their incident edges, dst-sharded) are partitioned
contiguously across the 8 cores (4096 nodes / 8 graphs per core); the
PLM/pooling branch is data-parallel over batch. BatchNorm statistics are
exchanged with a small AllGather + local combine; the 384-wide node
features y are AllGathered (fp16) for the edge gather.

Perf notes (CoreSim cost model):
- selector one-hots use fp16 iota/dstloc so TensorScalarPtr hits the 4x DVE
  mode (all-2-byte operands).
- BN statistics via the native bn_stats instruction (one pass, sum+var).
- dma_gather chunks span dst-window boundaries (16 tiles = 2048 descriptors
  per call; dynamic_dma_scratch_size=65536 gives a 4096-descriptor SWDGE
  ring) to amortize the ~1us fixed SWDGE overhead per gather.
- stat exchange = AllGather (15.6us) + 7 local adds, cheaper than AllReduce
  (28.2us) under the collective cost model.
"""
import sys
sys.path.insert(0, "/opt/trn_rl_repo")
from contextlib import ExitStack

import numpy as np
import ml_dtypes

import bass_rust as _br
import concourse.bacc as bacc
import concourse.bass as bass
import concourse.tile as tile
from concourse import mybir
from concourse._compat import cdiv

fp32 = mybir.dt.float32
bf16 = mybir.dt.float16  # "bf16" name kept; fp16 for 8x less quant noise
fp8 = mybir.dt.float8e4
i16 = mybir.dt.int16
AF = mybir.ActivationFunctionType
ALU = mybir.AluOpType
AX = mybir.AxisListType

NCORES = 8
B, S, E = 64, 512, 768
D = 384
NG = 512
N = B * NG              # 32768
NEDGE = 1048576
C = 3
NPC = N // NCORES       # 4096 nodes per core
GPC = B // NCORES       # 8 graphs per core
NW = NPC // 128         # 32 dst windows per core
NCOL = 512
NCH = NPC // NCOL       # 8 column chunks
EPS = 1e-5
GCHUNK = 8              # gather chunk: tiles (of 128 edges) per dma_gather;
                        # 1024 descs = the SWDGE ring; bigger wedges real HW
DPAD = 512              # y-table row bytes (fp8, padded): >=512B avoids the
                        # 2x DMA latency multiplier for sub-512B descriptors


# ---------------------------------------------------------------- BIR patch
def split_waits(nc):
    """walrus here supports ONE sync-wait per instruction; split extras onto
    NoOps inserted just before, on the same engine."""
    counter = 0
    for f in nc.m.functions:
        for bb in f.blocks:
            newlist, changed = [], False
            for inst in bb.instructions:
                si = inst.sync_info
                if si is not None and len(si.on_wait) > 1:
                    waits = list(si.on_wait)
                    for w in waits[:-1]:
                        counter += 1
                        nop = mybir.InstNoOp(name=f"I-WSPLIT-{counter}", ins=[], outs=[])
                        nop.engine = inst.engine
                        nop.sync_info = _br.SyncInfo(on_wait=[w], on_update=[])
                        newlist.append(nop)
                    inst.sync_info = _br.SyncInfo(
                        on_wait=[waits[-1]], on_update=list(si.on_update))
                    changed = True
                newlist.append(inst)
            if changed:
                bb.instructions = newlist


# ---------------------------------------------------------------- host prep
def _col3(v):
    """[384] -> [128, 3] column layout (feature f = c*128+p)."""
    return np.ascontiguousarray(v.reshape(3, 128).T).astype(np.float32)


def _col6(v):
    return np.ascontiguousarray(v.reshape(6, 128).T).astype(np.float32)


def _wchunks(w, kc, m):
    """[K, M] -> [128, kc, M] (k-chunk on partitions)."""
    K, M = w.shape
    assert K == kc * 128
    return np.ascontiguousarray(w.reshape(kc, 128, M).transpose(1, 0, 2))


def _wrap_idx(idx):
    """int16 idx array (len % 128 == 0) -> [128, len/16] dma_gather layout."""
    blk = idx.reshape(-1, 16).T  # [16, len/16]
    return np.ascontiguousarray(np.tile(blk, (8, 1)))


def preprocess(inputs):
    ei = np.asarray(inputs["edge_index"]).astype(np.int64)
    src_all = ei[0]
    dst_all = ei[1]

    # self-loops are NOT in the edge stream: deg counts them (+1) and the
    # dst owner adds y[n] to its aggregate at finalize time.
    deg = (np.bincount(dst_all, minlength=N) + 1).astype(np.float32)

    # src-sharded conv: core c owns edges whose src is in its node range and
    # accumulates partial sums over ALL 256 global dst windows; a bf16
    # ReduceScatter then sums and distributes rows back to their dst owners.
    # Tiles are packed contiguously across window boundaries (uniform
    # per-window edge budget ec = max over cores); boundary tiles run one
    # selector matmul per window they span.
    score = src_all // NPC
    gwin = dst_all >> 7
    order = np.lexsort((src_all, gwin, score))
    src_s = src_all[order]
    dst_s = dst_all[order]
    score_s = score[order]
    gwin_s = gwin[order]
    cstart = np.searchsorted(score_s, np.arange(NCORES + 1))

    NWG = N // 128                                        # 256 global windows
    cnts = np.zeros((NCORES, NWG), np.int64)
    for c in range(NCORES):
        cnts[c] = np.bincount(gwin_s[cstart[c]:cstart[c + 1]], minlength=NWG)
    ec = np.maximum(cnts.max(axis=0), 1).astype(np.int64)  # edge budget/window
    estart = np.zeros(NWG + 1, np.int64)
    np.cumsum(ec, out=estart[1:])
    Etot = int(estart[-1])
    T_total = cdiv(Etot, 128)
    EPAD = T_total * 128

    # uniform window id per stream position
    wstream = np.repeat(np.arange(NWG), ec)
    wstream = np.concatenate([wstream, np.full(EPAD - Etot, -1, np.int64)])
    # per-tile (window, column) spans, shared across cores
    ncols = 0
    tile_spans = []
    for t in range(T_total):
        ws = np.unique(wstream[t * 128:(t + 1) * 128])
        ws = [int(w) for w in ws if w >= 0]
        tile_spans.append([(w, ncols + i) for i, w in enumerate(ws)])
        ncols += len(ws)

    # masked node indices (2 per graph, ascending)
    mask = np.asarray(inputs["graph_masking"])
    sel = np.argsort(-mask, axis=1, kind="stable")[:, :2]  # top_k: ones, asc idx
    sel = np.sort(sel, axis=1)

    xT = np.zeros((D, N), np.float32)
    xT[:300] = np.asarray(inputs["x_nodes"]).T
    xT = xT.astype(np.float16)

    w1p = np.zeros((D, D), np.float32)
    w1p[:300] = np.asarray(inputs["w_pre1"])

    lastf = np.asarray(inputs["last_h"]).astype(np.float16)
    firstf = np.asarray(inputs["first_h"]).astype(np.float16)

    bng_g, bng_b = np.asarray(inputs["bng_g"]), np.asarray(inputs["bng_b"])
    bn_g, bn_b = np.asarray(inputs["bn_g"]), np.asarray(inputs["bn_b"])
    # vec columns [128, 72]: order documented here, mirrored on device
    cols = [
        _col3(np.asarray(inputs["b_pre1"])), _col3(np.asarray(inputs["b_pre2"])),
        _col3(np.asarray(inputs["b_post1"])), _col3(np.asarray(inputs["b_post2"])),
        _col6(np.asarray(inputs["b_cat"])),
        _col3(bng_g[0]), _col3(bng_b[0]), _col3(bng_g[1]), _col3(bng_b[1]),
        _col3(bng_g[4]), _col3(bng_b[4]), _col3(bng_g[5]), _col3(bng_b[5]),
        _col3(bng_g[6]), _col3(bng_b[6]),
        _col6(bn_g[0]), _col6(bn_b[0]), _col6(bn_g[1]), _col6(bn_b[1]),
    ]
    vecs = np.concatenate(cols, axis=1)  # [128, 3*4+6+3*10+6*4] = [128, 72]
    brow = np.zeros((1, 512), np.float32)
    brow[0, :D] = np.asarray(inputs["b_conv"])[2]
    brow[0, D:D + C] = np.asarray(inputs["b_out"])

    w_bf = {
        "w1": _wchunks(w1p, 3, D).astype(np.float16),
        "w2": _wchunks(np.asarray(inputs["w_pre2"]), 3, D).astype(np.float16),
        "wc": _wchunks(np.asarray(inputs["w_conv"])[2], 3, D).astype(np.float16),
        "wp1": _wchunks(np.asarray(inputs["w_post1"]), 3, D).astype(np.float16),
        "wp2": _wchunks(np.asarray(inputs["w_post2"]), 3, D).astype(np.float16),
    }
    wcat = _wchunks(np.asarray(inputs["w_cat"]), 6, E).astype(np.float16)
    wout = _wchunks(np.asarray(inputs["w_out"]), 6, C).astype(np.float32)

    in_maps = []
    for c in range(NCORES):
        n0 = c * NPC
        src_pad = np.zeros(EPAD, np.int64)
        dstloc = np.full(EPAD, -1.0, np.float32)
        base = cstart[c]
        wofs = np.zeros(NWG + 1, np.int64)
        np.cumsum(cnts[c], out=wofs[1:])
        for gw in range(NWG):
            a, b_ = base + wofs[gw], base + wofs[gw + 1]
            k = b_ - a
            pos = estart[gw]
            src_pad[pos:pos + k] = src_s[a:b_] - n0          # local src idx
            dstloc[pos:pos + k] = (dst_s[a:b_] - gw * 128).astype(np.float32)
        idx_w = _wrap_idx(src_pad.astype(np.int16))            # [128, T*8]
        # one dstloc column per (tile, window) span
        dcols = np.full((ncols, 128), -1.0, np.float32)
        for t in range(T_total):
            tsl = slice(t * 128, (t + 1) * 128)
            wv = wstream[tsl]
            dv = dstloc[tsl]
            for (w, col) in tile_spans[t]:
                dcols[col] = np.where(wv == w, dv, -1.0)
        dst_t = np.ascontiguousarray(dcols.T)                  # [128, ncols]

        deg_nm = np.ascontiguousarray(
            deg[n0:n0 + NPC].reshape(NW, 128).T)               # [128, 32]

        gidx = (sel[c * GPC:(c + 1) * GPC] +
                np.arange(c * GPC, (c + 1) * GPC)[:, None] * NG - n0)  # local
        gidx = gidx.reshape(-1).astype(np.int16)               # [16]
        gidx_w = np.zeros((128, 1), np.int16)
        gidx_w[:16, 0] = gidx
        gidx_w = np.tile(gidx_w[:16], (8, 1))

        m = {
            "lasth": np.ascontiguousarray(
                lastf[c * GPC:(c + 1) * GPC].reshape(GPC * S, E)),
            "firsth": np.ascontiguousarray(
                firstf[c * GPC:(c + 1) * GPC].reshape(GPC * S, E)),
            "xT": np.ascontiguousarray(
                xT.reshape(3, 128, N)[:, :, n0:n0 + NPC].transpose(1, 0, 2)
            ).reshape(128, 3 * NPC),
            "eidx": idx_w, "dstloc": dst_t, "deg": deg_nm,
            "vecs": vecs, "brow": brow, "gidx": gidx_w,
            "w1": w_bf["w1"].reshape(128, 3 * D),
            "w2": w_bf["w2"].reshape(128, 3 * D),
            "wc": w_bf["wc"].reshape(128, 3 * D),
            "wp1": w_bf["wp1"].reshape(128, 3 * D),
            "wp2": w_bf["wp2"].reshape(128, 3 * D),
            "wcat": wcat.reshape(128, 6 * E),
            "wout": wout.reshape(128, 6 * C),
        }
        in_maps.append(m)
    meta = (tuple(int(e) for e in ec),)
    return in_maps, meta


# ---------------------------------------------------------------- device
def build(meta, rep=1, taps=(), stage=99):
    ec = np.asarray(meta[0], np.int64)
    NWG = len(ec)
    estart = np.zeros(NWG + 1, np.int64)
    np.cumsum(ec, out=estart[1:])
    Etot = int(estart[-1])
    T_total = cdiv(Etot, 128)
    EPAD = T_total * 128
    wstream = np.repeat(np.arange(NWG), ec)
    wstream = np.concatenate([wstream, np.full(EPAD - Etot, -1, np.int64)])
    ncols = 0
    tile_spans = []
    for t in range(T_total):
        ws = np.unique(wstream[t * 128:(t + 1) * 128])
        ws = [int(w) for w in ws if w >= 0]
        tile_spans.append([(w, ncols + i) for i, w in enumerate(ws)])
        ncols += len(ws)
    wt_first = {w: estart[w] // 128 for w in range(NWG)}
    wt_last = {w: (estart[w + 1] - 1) // 128 for w in range(NWG)}

    nc = bacc.Bacc("TRN2")
    I = {}
    I["lasth"] = nc.dram_tensor("lasth", [GPC * S, E], bf16, kind="ExternalInput")
    I["firsth"] = nc.dram_tensor("firsth", [GPC * S, E], bf16, kind="ExternalInput")
    I["xT"] = nc.dram_tensor("xT", [128, 3 * NPC], bf16, kind="ExternalInput")
    I["eidx"] = nc.dram_tensor("eidx", [128, T_total * 8], i16, kind="ExternalInput")
    I["dstloc"] = nc.dram_tensor("dstloc", [128, ncols], fp32, kind="ExternalInput")
    I["deg"] = nc.dram_tensor("deg", [128, NW], fp32, kind="ExternalInput")
    I["vecs"] = nc.dram_tensor("vecs", [128, 72], fp32, kind="ExternalInput")
    I["brow"] = nc.dram_tensor("brow", [1, 512], fp32, kind="ExternalInput")
    I["gidx"] = nc.dram_tensor("gidx", [128, 1], i16, kind="ExternalInput")
    for w in ("w1", "w2", "wc", "wp1", "wp2"):
        I[w] = nc.dram_tensor(w, [128, 3 * D], bf16, kind="ExternalInput")
    I["wcat"] = nc.dram_tensor("wcat", [128, 6 * E], bf16, kind="ExternalInput")
    I["wout"] = nc.dram_tensor("wout", [128, 6 * C], fp32, kind="ExternalInput")
    outT = nc.dram_tensor("outT", [C, GPC], fp32, kind="ExternalOutput")
    tap_outs = {}

    grp = [list(range(NCORES))]

    with tile.TileContext(nc) as tc, ExitStack() as ctx:
        const = ctx.enter_context(tc.tile_pool(name="const", bufs=1))
        big = ctx.enter_context(tc.tile_pool(name="big", bufs=2))
        gpool = ctx.enter_context(tc.tile_pool(name="gath", bufs=2))
        spool = ctx.enter_context(tc.tile_pool(name="small", bufs=2))
        selp = ctx.enter_context(tc.tile_pool(name="sel", bufs=8))
        hspool = ctx.enter_context(tc.tile_pool(name="hs", bufs=8))
        cinp = ctx.enter_context(tc.tile_pool(name="cin", bufs=1))
        mmps = ctx.enter_context(tc.tile_pool(name="mmps", bufs=2, space="PSUM"))
        cvps = ctx.enter_context(tc.tile_pool(name="cvps", bufs=2, space="PSUM"))
        trps = ctx.enter_context(tc.tile_pool(name="trps", bufs=2, space="PSUM"))
        typs = ctx.enter_context(tc.tile_pool(name="typs", bufs=2, space="PSUM"))
        dram = ctx.enter_context(tc.tile_pool(name="dram", bufs=1, space="DRAM"))

        # ---------------- constants
        iota = const.tile([128, 128], bf16)
        nc.gpsimd.iota(iota[:], pattern=[[1, 128]], base=0, channel_multiplier=0,
                       allow_small_or_imprecise_dtypes=True)
        pidx = const.tile([128, 1], fp32)  # partition index column
        nc.gpsimd.iota(pidx[:], pattern=[[0, 1]], base=0, channel_multiplier=1,
                       allow_small_or_imprecise_dtypes=True)
        ident = const.tile([128, 128], bf16)
        nc.vector.tensor_scalar(ident[:], iota[:], pidx[:], None, ALU.is_equal)

        idx_t = const.tile([128, T_total * 8], i16)
        nc.sync.dma_start(idx_t[:], I["eidx"][:])
        dst_t = const.tile([128, ncols], fp32)
        nc.sync.dma_start(dst_t[:], I["dstloc"][:])
        deg_t = const.tile([128, NW], fp32)
        nc.sync.dma_start(deg_t[:], I["deg"][:])
        vecs = const.tile([128, 72], fp32)
        nc.sync.dma_start(vecs[:], I["vecs"][:])
        brow = const.tile([1, 512], fp32)
        nc.sync.dma_start(brow[:], I["brow"][:])
        gidx_t = const.tile([128, 1], i16)
        nc.sync.dma_start(gidx_t[:], I["gidx"][:])
        W = {}
        for w in ("w1", "w2", "wc", "wp1", "wp2"):
            W[w] = const.tile([128, 3, D], bf16, name=f"W_{w}", tag=f"W_{w}")
            nc.sync.dma_start(W[w][:], I[w][:].rearrange("p (k m) -> p k m", k=3))
        wcat = const.tile([128, 6, E], bf16)
        nc.sync.dma_start(wcat[:], I["wcat"][:].rearrange("p (k m) -> p k m", k=6))
        wout = const.tile([128, 6, C], fp32)
        nc.sync.dma_start(wout[:], I["wout"][:].rearrange("p (k m) -> p k m", k=6))
        bconv_bc = const.tile([128, D], fp32)
        nc.gpsimd.partition_broadcast(bconv_bc[:], brow[0:1, 0:D])
        ones8 = const.tile([1, GPC], fp32)
        nc.vector.memset(ones8[:], 1.0)
        onescol = const.tile([128, 1], bf16)
        nc.vector.memset(onescol[:], 1.0)

        # vec column offsets
        VO = {}
        off = 0
        for name, w_ in [("b1", 3), ("b2", 3), ("bp1", 3), ("bp2", 3), ("bcat", 6),
                         ("g0", 3), ("be0", 3), ("g1", 3), ("be1", 3),
                         ("g4", 3), ("be4", 3), ("g5", 3), ("be5", 3),
                         ("g6", 3), ("be6", 3),
                         ("gc0", 6), ("bc0", 6), ("gc1", 6), ("bc1", 6)]:
            VO[name] = (off, w_)
            off += w_
        def vcol(name):
            o, w_ = VO[name]
            return vecs[:, o:o + w_]

        # deg^-1/2
        dinv = const.tile([128, NW], fp32)
        nc.scalar.sqrt(dinv[:], deg_t[:])
        nc.vector.reciprocal(dinv[:], dinv[:])

        def tap(name, ap):
            if name not in taps:
                return
            t_ = nc.dram_tensor(f"tap_{name}", list(ap.shape), ap.dtype,
                                kind="ExternalOutput")
            tap_outs[name] = t_
            nc.sync.dma_start(t_[:], ap)

        def finish_early(src_ap, width=GPC):
            fin0 = spool.tile([C, GPC], fp32, name="fin0", tag="fin")
            nc.vector.memset(fin0[:], 0.0)
            nc.vector.tensor_scalar(fin0[:, 0:width], src_ap, 1.0, None, ALU.mult)
            nc.sync.dma_start(outT[:], fin0[:])

        for _rep in range(rep):
            # ---------------- helpers
            def _combine_bn_raw(raw, st, m_chunks):
                """raw [128, m, NCH, 6] bn_stats outputs -> st [128, m, 2]
                holding (sum/256, sumsq/256) per feature."""
                Ev = raw[:, :, :, 1]
                Vv = raw[:, :, :, 2]
                Ov = raw[:, :, :, 4]
                Wv = raw[:, :, :, 5]
                e2 = spool.tile([128, m_chunks, NCH], fp32, name="e2", tag="cb_e2")
                o2 = spool.tile([128, m_chunks, NCH], fp32, name="o2", tag="cb_o2")
                vv = spool.tile([128, m_chunks, NCH], fp32, name="vv", tag="cb_vv")
                ss = spool.tile([128, m_chunks, NCH], fp32, name="ss", tag="cb_ss")
                nc.vector.tensor_mul(e2[:], Ev, Ev)
                nc.vector.tensor_mul(o2[:], Ov, Ov)
                nc.vector.tensor_add(e2[:], e2[:], o2[:])
                nc.vector.tensor_tensor(vv[:], Vv, Wv, ALU.add)
                nc.vector.scalar_tensor_tensor(
                    vv[:], vv[:], 1.0 / 256, e2[:], op0=ALU.mult, op1=ALU.add)
                nc.vector.tensor_tensor(ss[:], Ev, Ov, ALU.add)
                nc.vector.reduce_sum(st[:, :, 0:1], ss[:], axis=AX.X)
                nc.vector.reduce_sum(st[:, :, 1:2], vv[:], axis=AX.X)

            def bn_stats_pack(src_view, m_chunks, tag):
                """src_view(m) -> [128, NCH, NCOL] bf16 view; returns
                [128, m_chunks*2] (sum/256, sumsq/256) stat tile."""
                raw = spool.tile([128, m_chunks, NCH, 6], fp32, tag=tag + "_bs")
                for m in range(m_chunks):
                    sv = src_view(m)
                    for j in range(NCH):
                        nc.vector.bn_stats(raw[:, m, j:j + 1, :], sv[:, j, :])
                st = spool.tile([128, m_chunks, 2], fp32, tag=tag + "_st")
                _combine_bn_raw(raw, st, m_chunks)
                return st

            def ag_exchange(st_ap, width, tag):
                """AllGather the [128, width] stat tile; sum over ranks."""
                cin = dram.tile([128, width], fp32, tag=tag + "_ci")
                cout = dram.tile([NCORES * 128, width], fp32, tag=tag + "_co")
                nc.sync.dma_start(cin[:], st_ap)
                nc.gpsimd.collective_compute(
                    "AllGather", ALU.bypass, replica_groups=grp,
                    ins=[cin[:]], outs=[cout[:]])
                allsb = spool.tile([128, NCORES, width], fp32, tag=tag + "_as")
                nc.sync.dma_start(
                    allsb[:], cout[:].rearrange("(r p) w -> p r w", p=128))
                red = spool.tile([128, width], fp32, tag=tag + "_rd")
                nc.vector.tensor_tensor(red[:], allsb[:, 0], allsb[:, 1], ALU.add)
                for r in range(2, NCORES):
                    nc.vector.tensor_tensor(red[:], red[:], allsb[:, r], ALU.add)
                return red

            def bn_coeffs(red, m_chunks, count, gname, bname, tag):
                """red [128, 2*m]: per-feature sum/sumsq -> gp, bp [128, m]"""
                gp = spool.tile([128, m_chunks], fp32, tag=tag + "_gp")
                bp = spool.tile([128, m_chunks], fp32, tag=tag + "_bp")
                mu = spool.tile([128, m_chunks], fp32, tag=tag + "_mu")
                var = spool.tile([128, m_chunks], fp32, tag=tag + "_va")
                inv_n = 1.0 / count
                sview = red.rearrange("p (m two) -> p m two", two=2)
                nc.vector.tensor_scalar(mu[:], sview[:, :, 0], inv_n, None, ALU.mult)
                nc.vector.tensor_scalar(var[:], sview[:, :, 1], inv_n, None, ALU.mult)
                musq = spool.tile([128, m_chunks], fp32, tag=tag + "_ms")
                nc.vector.tensor_mul(musq[:], mu[:], mu[:])
                nc.vector.tensor_tensor(var[:], var[:], musq[:], ALU.subtract)
                nc.vector.tensor_scalar(var[:], var[:], EPS, None, ALU.add)
                nc.scalar.sqrt(var[:], var[:])
                nc.vector.reciprocal(var[:], var[:])          # 1/sigma
                nc.vector.tensor_mul(gp[:], vcol(gname), var[:])
                nc.vector.tensor_mul(bp[:], gp[:], mu[:])
                nc.vector.tensor_tensor(bp[:], vcol(bname), bp[:], ALU.subtract)
                return gp, bp

            def bn_apply(x_t, gp, bp, m_chunks, width):
                for m in range(m_chunks):
                    nc.vector.tensor_scalar(
                        x_t[:, m, :width], x_t[:, m, :width],
                        gp[:, m:m + 1], bp[:, m:m + 1], ALU.mult, ALU.add)

            def mlp_layer(x_t, w_t, bias_col, tag):
                """x_t [128,3,NPC] bf16 -> relu(x@W + b) bf16 [128,3,NPC] + stats"""
                out = big.tile([128, 3, NPC], bf16, tag="big")
                for m in range(3):
                    for j in range(NCH):
                        sl = slice(j * NCOL, (j + 1) * NCOL)
                        ps = mmps.tile([128, NCOL], fp32, tag="mm")
                        for k in range(3):
                            nc.tensor.matmul(
                                ps[:], lhsT=w_t[:, k, m * 128:(m + 1) * 128],
                                rhs=x_t[:, k, sl], start=(k == 0), stop=(k == 2))
                        nc.scalar.activation(out[:, m, sl], ps[:], AF.Relu,
                                             bias=bias_col[:, m:m + 1])
                st = bn_stats_pack(
                    lambda m: out[:, m, :].rearrange("p (j n) -> p j n", n=NCOL),
                    3, tag)
                return out, st

            # ---------------- pre-MLPs
            xT_t = big.tile([128, 3, NPC], bf16, tag="big")
            nc.sync.dma_start(xT_t[:], I["xT"][:].rearrange("p (k n) -> p k n", k=3))

            x1, st1 = mlp_layer(xT_t, W["w1"], vcol("b1"), "l1")
            red1 = ag_exchange(st1[:], 6, "ar1")
            gp1, bp1 = bn_coeffs(red1, 3, N // 256, "g0", "be0", "bn1")
            bn_apply(x1, gp1, bp1, 3, NPC)
            tap("x1", x1[:])

            x2, st2 = mlp_layer(x1, W["w2"], vcol("b2"), "l2")
            red2 = ag_exchange(st2[:], 6, "ar2")
            gp2, bp2 = bn_coeffs(red2, 3, N // 256, "g1", "be1", "bn2")
            bn_apply(x2, gp2, bp2, 3, NPC)
            tap("x2", x2[:])
            if stage <= 2:
                finish_early(x2[0:C, 0, 0:GPC])
                continue

            # ---------------- z = x2 @ wc ; y = z * dinv (node-major, fp8
            # rows padded to DPAD bytes so each gather descriptor is >=512B)
            y_slice = dram.tile([NPC, D], bf16, tag="y_slice")
            z_sb = big.tile([128, 3, NPC], bf16, tag="big")
            for m in range(3):
                for j in range(NCH):
                    sl = slice(j * NCOL, (j + 1) * NCOL)
                    ps = mmps.tile([128, NCOL], fp32, tag="mm")
                    for k in range(3):
                        nc.tensor.matmul(
                            ps[:], lhsT=W["wc"][:, k, m * 128:(m + 1) * 128],
                            rhs=x2[:, k, sl], start=(k == 0), stop=(k == 2))
                    nc.scalar.activation(z_sb[:, m, sl], ps[:], AF.Copy)
            for w in range(NW):
                ywin = spool.tile([128, D], bf16, tag="ywin")
                for m in range(3):
                    trp = trps.tile([128, 128], bf16, tag="tr")
                    nc.tensor.transpose(
                        trp[:], z_sb[:, m, w * 128:(w + 1) * 128], ident[:])
                    nc.vector.tensor_scalar(
                        ywin[:, m * 128:(m + 1) * 128], trp[:],
                        dinv[:, w:w + 1], None, ALU.mult)
                nc.sync.dma_start(y_slice[w * 128:(w + 1) * 128, :], ywin[:])

            # ---------------- conv: per-core partial sums over ALL 256 global
            # dst windows (gathering from the LOCAL y_slice only), written
            # bf16 to pagg, then summed+sharded with one ReduceScatter.
            pagg = dram.tile([N, D], bf16, tag="pagg")
            ps_win = {}

            def partial_flush(gw):
                ps_c = ps_win.pop(gw)
                pb = spool.tile([128, D], bf16, name="pb", tag="pb")
                nc.scalar.activation(pb[:], ps_c[:], AF.Copy)
                nc.sync.dma_start(pagg[gw * 128:(gw + 1) * 128, :], pb[:])

            t = 0
            while t < T_total:
                cn = min(GCHUNK, T_total - t)
                gt = gpool.tile([128, GCHUNK * D], bf16, tag="g")
                nc.gpsimd.dma_gather(
                    out_ap=gt[:, :cn * D].rearrange("p (t f) -> p t f", f=D),
                    in_ap=y_slice[:],
                    idxs_ap=idx_t[:, t * 8:(t + cn) * 8],
                    num_idxs=cn * 128, num_idxs_reg=cn * 128, elem_size=D)
                gv = gt[:, :cn * D].rearrange("p (t f) -> p t f", f=D)
                for tl in range(cn):
                    tg = t + tl
                    for (w, col) in tile_spans[tg]:
                        if tg == wt_first[w]:
                            ps_win[w] = cvps.tile([128, D], fp32, name="ps_c",
                                                  tag="cv")
                        sel = selp.tile([128, 128], bf16, tag="sel")
                        nc.vector.tensor_scalar(
                            sel[:], iota[:], dst_t[:, col:col + 1],
                            None, ALU.is_equal)
                        last = (tg == wt_last[w])
                        nc.tensor.matmul(ps_win[w][:], lhsT=sel[:],
                                         rhs=gv[:, tl, :],
                                         start=(tg == wt_first[w]), stop=last)
                        if last:
                            partial_flush(w)
                t += cn

            conv_in = dram.tile([NPC, D], bf16, tag="conv_in")
            nc.gpsimd.collective_compute(
                "ReduceScatter", ALU.add, replica_groups=grp,
                ins=[pagg[:]], outs=[conv_in[:]])

            # ---------------- sentence branch (overlaps the ReduceScatter)
            HsT = spool.tile([128, 6, GPC], fp32, tag="HsT")
            for b in range(GPC):
                ps_ht = typs.tile([128, GPC], fp32, tag="tiny")
                hts = []
                for hsrc in (I["lasth"], I["firsth"]):
                    for sc in range(4):
                        ht = hspool.tile([128, E], bf16, name="ht", tag="ht")
                        nc.sync.dma_start(
                            ht[:], hsrc[b * S + sc * 128:b * S + (sc + 1) * 128, :])
                        hts.append(ht)
                for m in range(6):
                    for i, ht in enumerate(hts):
                        nc.tensor.matmul(
                            ps_ht[:, m:m + 1],
                            lhsT=ht[:, m * 128:(m + 1) * 128],
                            rhs=onescol[:],
                            start=(i == 0), stop=(i == 7))
                nc.vector.tensor_scalar(
                    HsT[:, :, b], ps_ht[:, 0:6],
                    1.0 / (2 * S), None, ALU.mult)
            tap("hsT", HsT[:])
            if stage <= 3:
                finish_early(HsT[0:C, 0, 0:GPC])
                continue

            # ---------------- own-shard finalize: scale/bias/relu/transpose
            convT = big.tile([128, 3, NPC], bf16, tag="big")
            cin_sb = cinp.tile([128, NW, D], bf16, tag="cin")
            nc.sync.dma_start(
                cin_sb[:], conv_in[:].rearrange("(w p) f -> p w f", p=128))
            ysb = cinp.tile([128, NW, D], bf16, tag="ysb")
            nc.sync.dma_start(
                ysb[:], y_slice[:].rearrange("(w p) f -> p w f", p=128))
            raw4 = spool.tile([128, 3, NCH, 6], fp32, tag="l4_bs")
            for w in range(NW):
                # fold the self-loop message y[n] into the aggregate
                agg = spool.tile([128, D], bf16, name="agg", tag="agg")
                nc.vector.tensor_add(agg[:], cin_sb[:, w, :], ysb[:, w, :])
                winf = spool.tile([128, D], fp32, tag="winf")
                nc.vector.scalar_tensor_tensor(
                    winf[:], agg[:], dinv[:, w:w + 1], bconv_bc[:],
                    op0=ALU.mult, op1=ALU.add)
                winb = spool.tile([128, D], bf16, tag="winb")
                nc.scalar.activation(winb[:], winf[:], AF.Relu)
                for m in range(3):
                    trp = trps.tile([128, 128], bf16, tag="tr")
                    nc.tensor.transpose(trp[:], winb[:, m * 128:(m + 1) * 128],
                                        ident[:])
                    nc.vector.tensor_copy(convT[:, m, w * 128:(w + 1) * 128],
                                          trp[:])
                if w % 4 == 3:
                    j = w // 4
                    for m in range(3):
                        nc.vector.bn_stats(raw4[:, m, j:j + 1, :],
                                           convT[:, m, j * NCOL:(j + 1) * NCOL])

            st4 = spool.tile([128, 3, 2], fp32, tag="st4")
            _combine_bn_raw(raw4, st4, 3)
            red4 = ag_exchange(st4[:], 6, "ar4")
            gp4, bp4 = bn_coeffs(red4, 3, N // 256, "g4", "be4", "bn4")
            bn_apply(convT, gp4, bp4, 3, NPC)
            tap("convT", convT[:])
            if stage <= 4:
                finish_early(convT[0:C, 0, 0:GPC])
                continue

            # ---------------- post MLPs
            p1, st5 = mlp_layer(convT, W["wp1"], vcol("bp1"), "l5")
            red5 = ag_exchange(st5[:], 6, "ar5")
            gp5, bp5 = bn_coeffs(red5, 3, N // 256, "g5", "be5", "bn5")
            bn_apply(p1, gp5, bp5, 3, NPC)

            # post2: row-major bf16 to DRAM (pre-BN); stats collected
            p2_dram = dram.tile([NPC, D], bf16, tag="p2")
            raw6 = spool.tile([128, 3, NCH, 6], fp32, tag="p2_bs")
            for j in range(NCH):
                sl = slice(j * NCOL, (j + 1) * NCOL)
                p2c = spool.tile([128, 3, NCOL], bf16, tag="p2c")
                for m in range(3):
                    ps = mmps.tile([128, NCOL], fp32, tag="mm")
                    for k in range(3):
                        nc.tensor.matmul(
                            ps[:], lhsT=W["wp2"][:, k, m * 128:(m + 1) * 128],
                            rhs=p1[:, k, sl], start=(k == 0), stop=(k == 2))
                    nc.scalar.activation(p2c[:, m, :], ps[:], AF.Relu,
                                         bias=vcol("bp2")[:, m:m + 1])
                    nc.vector.bn_stats(raw6[:, m, j:j + 1, :], p2c[:, m, :])
                for nb in range(NCOL // 128):
                    rmw = spool.tile([128, D], bf16, tag="rmw")
                    for m in range(3):
                        trp = trps.tile([128, 128], bf16, tag="tr")
                        nc.tensor.transpose(
                            trp[:], p2c[:, m, nb * 128:(nb + 1) * 128], ident[:])
                        nc.vector.tensor_copy(rmw[:, m * 128:(m + 1) * 128], trp[:])
                    nc.sync.dma_start(
                        p2_dram[j * NCOL + nb * 128:j * NCOL + (nb + 1) * 128, :],
                        rmw[:])
            # combine raw6 -> st6 [128, 3, 2] (sum/256, sumsq/256)
            st6 = spool.tile([128, 3, 2], fp32, tag="st6")
            _combine_bn_raw(raw6, st6, 3)
            red6 = ag_exchange(st6[:], 6, "ar6")
            gp6, bp6 = bn_coeffs(red6, 3, N // 256, "g6", "be6", "bn6")
            if stage <= 5:
                finish_early(red6[0:C, 0:6], width=6)
                continue

            # ---------------- masked-node gather -> flT [128, 3, 16] fp32 (BN6'd)
            gth = spool.tile([128, D], bf16, tag="gth")
            nc.gpsimd.dma_gather(
                out_ap=gth[:].rearrange("p (t f) -> p t f", f=D),
                in_ap=p2_dram[:], idxs_ap=gidx_t[:],
                num_idxs=16, num_idxs_reg=16, elem_size=D)
            flT = spool.tile([128, 3, 16], bf16, tag="flT")
            for m in range(3):
                trp_full = trps.tile([128, 128], bf16, tag="tr")
                trp = trp_full[:, 0:16]
                nc.tensor.matmul(trp, lhsT=gth[0:16, m * 128:(m + 1) * 128],
                                 rhs=ident[0:16, 0:16], is_transpose=True)
                nc.vector.tensor_scalar(flT[:, m, :], trp,
                                        gp6[:, m:m + 1], bp6[:, m:m + 1],
                                        ALU.mult, ALU.add)
            tap("flT", flT[:])

            # ---------------- tail: outc, H_sent BN, att, out
            outcT = spool.tile([128, 6, GPC], fp32, tag="outcT")
            for m in range(6):
                ps_o = typs.tile([128, GPC], fp32, tag="tiny")
                for k in range(6):
                    kc, kj = k % 3, k // 3
                    nc.tensor.matmul(
                        ps_o[:], lhsT=wcat[:, k, m * 128:(m + 1) * 128],
                        rhs=flT[:, kc, kj::2], start=(k == 0), stop=(k == 5))
                nc.scalar.activation(outcT[:, m, :], ps_o[:], AF.Relu,
                                     bias=vcol("bcat")[:, m:m + 1])
            stt = spool.tile([128, 24], fp32, tag="stt")
            for m in range(6):
                nc.vector.reduce_sum(stt[:, 2 * m:2 * m + 1], outcT[:, m, :], axis=AX.X)
                sq = spool.tile([128, GPC], fp32, tag="ttsq")
                nc.scalar.square(sq[:], outcT[:, m, :])
                nc.vector.reduce_sum(stt[:, 2 * m + 1:2 * m + 2], sq[:], axis=AX.X)
                nc.vector.reduce_sum(stt[:, 12 + 2 * m:13 + 2 * m], HsT[:, m, :], axis=AX.X)
                nc.scalar.square(sq[:], HsT[:, m, :])
                nc.vector.reduce_sum(stt[:, 13 + 2 * m:14 + 2 * m], sq[:], axis=AX.X)
            redt = ag_exchange(stt[:], 24, "art")
            gpc_, bpc_ = bn_coeffs(redt[:, 0:12], 6, B, "gc0", "bc0", "bnc")
            gph, bph = bn_coeffs(redt[:, 12:24], 6, B, "gc1", "bc1", "bnh")
            attT = spool.tile([128, 6, GPC], fp32, tag="attT")
            for m in range(6):
                nc.vector.tensor_scalar(attT[:, m, :], HsT[:, m, :],
                                        gph[:, m:m + 1], bph[:, m:m + 1],
                                        ALU.mult, ALU.add)
                nc.vector.tensor_scalar(outcT[:, m, :], outcT[:, m, :],
                                        gpc_[:, m:m + 1], bpc_[:, m:m + 1],
                                        ALU.mult, ALU.add)
                nc.vector.tensor_add(attT[:, m, :], attT[:, m, :], outcT[:, m, :])
            ps_ft = typs.tile([128, GPC], fp32, tag="tiny")
            ps_f = ps_ft[0:C, :]
            for k in range(6):
                kc, kj = k % 3, k // 3
                nc.tensor.matmul(ps_f, lhsT=wout[:, k, :], rhs=attT[:, k, :],
                                 start=(k == 0), stop=False)
            nc.tensor.matmul(ps_f, lhsT=brow[0:1, D:D + C], rhs=ones8[:],
                             start=False, stop=True)
            fin = spool.tile([C, GPC], fp32, tag="fin")
            nc.vector.tensor_copy(fin[:], ps_f)
            nc.sync.dma_start(outT[:], fin[:])

    nc.compile()
    return nc, tap_outs


# ---------------------------------------------------------------- entry
_CACHE = {}


def _get_compiled(meta):
    key = meta
    if key not in _CACHE:
        nc, _ = build(meta)
        split_waits(nc)
        _CACHE[key] = nc
    return _CACHE[key]


def kernel(**inputs):
    in_maps, meta = preprocess(inputs)
    nc = _get_compiled(meta)
    from concourse import bass2jax
    results = bass2jax.run_bass_via_pjrt(nc, in_maps, n_cores=NCORES)
    out = np.concatenate([results[c]["outT"].T for c in range(NCORES)], axis=0)
    return out.astype(np.float32)


# revision 47
# speedup vs baseline: 1.1573x; 1.0186x over previous
"""Trainium2 Bass kernel for nn_BaselineModelWithGNN (8-core SPMD).

Self-contained: hardcodes shapes/sharding; builds, compiles and runs the Bass
program on 8 NeuronCores via the axon PJRT path.

Key observation: the reference applies each of the 3 GCN convs to the same
input x and overwrites `out`, so only conv i=2 (w_conv[2], b_conv[2],
bng[4]) affects the result — one conv is computed.

Sharding: nodes (and their incident edges, dst-sharded) are partitioned
contiguously across the 8 cores (4096 nodes / 8 graphs per core); the
PLM/pooling branch is data-parallel over batch. BatchNorm statistics are
exchanged with a small AllGather + local combine; the 384-wide node
features y are AllGathered (fp16) for the edge gather.

Perf notes (CoreSim cost model):
- selector one-hots use fp16 iota/dstloc so TensorScalarPtr hits the 4x DVE
  mode (all-2-byte operands).
- BN statistics via the native bn_stats instruction (one pass, sum+var).
- dma_gather chunks span dst-window boundaries (16 tiles = 2048 descriptors
  per call; dynamic_dma_scratch_size=65536 gives a 4096-descriptor SWDGE
  ring) to amortize the ~1us fixed SWDGE overhead per gather.
- stat exchange = AllGather (15.6us) + 7 local adds, cheaper than AllReduce
  (28.2us) under the collective cost model.
"""
import sys
sys.path.insert(0, "/opt/trn_rl_repo")
from contextlib import ExitStack

import numpy as np
import ml_dtypes

import bass_rust as _br
import concourse.bacc as bacc
import concourse.bass as bass
import concourse.tile as tile
from concourse import mybir
from concourse._compat import cdiv

fp32 = mybir.dt.float32
bf16 = mybir.dt.float16  # "bf16" name kept; fp16 for 8x less quant noise
fp8 = mybir.dt.float8e4
i16 = mybir.dt.int16
AF = mybir.ActivationFunctionType
ALU = mybir.AluOpType
AX = mybir.AxisListType

NCORES = 8
B, S, E = 64, 512, 768
D = 384
NG = 512
N = B * NG              # 32768
NEDGE = 1048576
C = 3
NPC = N // NCORES       # 4096 nodes per core
GPC = B // NCORES       # 8 graphs per core
NW = NPC // 128         # 32 dst windows per core
NCOL = 512
NCH = NPC // NCOL       # 8 column chunks
EPS = 1e-5
GCHUNK = 8              # gather chunk: tiles (of 128 edges) per dma_gather;
                        # 1024 descs = the SWDGE ring; bigger wedges real HW
DPAD = 512              # y-table row bytes (fp8, padded): >=512B avoids the
                        # 2x DMA latency multiplier for sub-512B descriptors


# ---------------------------------------------------------------- BIR patch
def split_waits(nc):
    """walrus here supports ONE sync-wait per instruction; split extras onto
    NoOps inserted just before, on the same engine."""
    counter = 0
    for f in nc.m.functions:
        for bb in f.blocks:
            newlist, changed = [], False
            for inst in bb.instructions:
                si = inst.sync_info
                if si is not None and len(si.on_wait) > 1:
                    waits = list(si.on_wait)
                    for w in waits[:-1]:
                        counter += 1
                        nop = mybir.InstNoOp(name=f"I-WSPLIT-{counter}", ins=[], outs=[])
                        nop.engine = inst.engine
                        nop.sync_info = _br.SyncInfo(on_wait=[w], on_update=[])
                        newlist.append(nop)
                    inst.sync_info = _br.SyncInfo(
                        on_wait=[waits[-1]], on_update=list(si.on_update))
                    changed = True
                newlist.append(inst)
            if changed:
                bb.instructions = newlist


# ---------------------------------------------------------------- host prep
def _col3(v):
    """[384] -> [128, 3] column layout (feature f = c*128+p)."""
    return np.ascontiguousarray(v.reshape(3, 128).T).astype(np.float32)


def _col6(v):
    return np.ascontiguousarray(v.reshape(6, 128).T).astype(np.float32)


def _wchunks(w, kc, m):
    """[K, M] -> [128, kc, M] (k-chunk on partitions)."""
    K, M = w.shape
    assert K == kc * 128
    return np.ascontiguousarray(w.reshape(kc, 128, M).transpose(1, 0, 2))


def _wrap_idx(idx):
    """int16 idx array (len % 128 == 0) -> [128, len/16] dma_gather layout."""
    blk = idx.reshape(-1, 16).T  # [16, len/16]
    return np.ascontiguousarray(np.tile(blk, (8, 1)))


def preprocess(inputs):
    ei = np.asarray(inputs["edge_index"]).astype(np.int64)
    src_all = ei[0]
    dst_all = ei[1]

    # self-loops are NOT in the edge stream: deg counts them (+1) and the
    # dst owner adds y[n] to its aggregate at finalize time.
    deg = (np.bincount(dst_all, minlength=N) + 1).astype(np.float32)

    # src-sharded conv: core c owns edges whose src is in its node range and
    # accumulates partial sums over ALL 256 global dst windows; a bf16
    # ReduceScatter then sums and distributes rows back to their dst owners.
    # Tiles are packed contiguously across window boundaries (uniform
    # per-window edge budget ec = max over cores); boundary tiles run one
    # selector matmul per window they span.
    score = src_all // NPC
    gwin = dst_all >> 7
    order = np.lexsort((src_all, gwin, score))
    src_s = src_all[order]
    dst_s = dst_all[order]
    score_s = score[order]
    gwin_s = gwin[order]
    cstart = np.searchsorted(score_s, np.arange(NCORES + 1))

    NWG = N // 128                                        # 256 global windows
    cnts = np.zeros((NCORES, NWG), np.int64)
    for c in range(NCORES):
        cnts[c] = np.bincount(gwin_s[cstart[c]:cstart[c + 1]], minlength=NWG)
    ec = np.maximum(cnts.max(axis=0), 1).astype(np.int64)  # edge budget/window
    estart = np.zeros(NWG + 1, np.int64)
    np.cumsum(ec, out=estart[1:])
    Etot = int(estart[-1])
    T_total = cdiv(Etot, 128)
    EPAD = T_total * 128

    # uniform window id per stream position
    wstream = np.repeat(np.arange(NWG), ec)
    wstream = np.concatenate([wstream, np.full(EPAD - Etot, -1, np.int64)])
    # per-tile (window, column) spans, shared across cores
    ncols = 0
    tile_spans = []
    for t in range(T_total):
        ws = np.unique(wstream[t * 128:(t + 1) * 128])
        ws = [int(w) for w in ws if w >= 0]
        tile_spans.append([(w, ncols + i) for i, w in enumerate(ws)])
        ncols += len(ws)

    # masked node indices (2 per graph, ascending)
    mask = np.asarray(inputs["graph_masking"])
    sel = np.argsort(-mask, axis=1, kind="stable")[:, :2]  # top_k: ones, asc idx
    sel = np.sort(sel, axis=1)

    xT = np.zeros((D, N), np.float32)
    xT[:300] = np.asarray(inputs["x_nodes"]).T
    xT = xT.astype(np.float16)

    w1p = np.zeros((D, D), np.float32)
    w1p[:300] = np.asarray(inputs["w_pre1"])

    lastf = np.asarray(inputs["last_h"]).astype(np.float16)
    firstf = np.asarray(inputs["first_h"]).astype(np.float16)

    bng_g, bng_b = np.asarray(inputs["bng_g"]), np.asarray(inputs["bng_b"])
    bn_g, bn_b = np.asarray(inputs["bn_g"]), np.asarray(inputs["bn_b"])
    # vec columns [128, 72]: order documented here, mirrored on device
    cols = [
        _col3(np.asarray(inputs["b_pre1"])), _col3(np.asarray(inputs["b_pre2"])),
        _col3(np.asarray(inputs["b_post1"])), _col3(np.asarray(inputs["b_post2"])),
        _col6(np.asarray(inputs["b_cat"])),
        _col3(bng_g[0]), _col3(bng_b[0]), _col3(bng_g[1]), _col3(bng_b[1]),
        _col3(bng_g[4]), _col3(bng_b[4]), _col3(bng_g[5]), _col3(bng_b[5]),
        _col3(bng_g[6]), _col3(bng_b[6]),
        _col6(bn_g[0]), _col6(bn_b[0]), _col6(bn_g[1]), _col6(bn_b[1]),
    ]
    vecs = np.concatenate(cols, axis=1)  # [128, 3*4+6+3*10+6*4] = [128, 72]
    brow = np.zeros((1, 512), np.float32)
    brow[0, :D] = np.asarray(inputs["b_conv"])[2]
    brow[0, D:D + C] = np.asarray(inputs["b_out"])

    w_bf = {
        "w1": _wchunks(w1p, 3, D).astype(np.float16),
        "w2": _wchunks(np.asarray(inputs["w_pre2"]), 3, D).astype(np.float16),
        "wc": _wchunks(np.asarray(inputs["w_conv"])[2], 3, D).astype(np.float16),
        "wp1": _wchunks(np.asarray(inputs["w_post1"]), 3, D).astype(np.float16),
        "wp2": _wchunks(np.asarray(inputs["w_post2"]), 3, D).astype(np.float16),
    }
    wcat = _wchunks(np.asarray(inputs["w_cat"]), 6, E).astype(np.float16)
    wout = _wchunks(np.asarray(inputs["w_out"]), 6, C).astype(np.float32)

    in_maps = []
    for c in range(NCORES):
        n0 = c * NPC
        src_pad = np.zeros(EPAD, np.int64)
        dstloc = np.full(EPAD, -1.0, np.float32)
        base = cstart[c]
        wofs = np.zeros(NWG + 1, np.int64)
        np.cumsum(cnts[c], out=wofs[1:])
        for gw in range(NWG):
            a, b_ = base + wofs[gw], base + wofs[gw + 1]
            k = b_ - a
            pos = estart[gw]
            src_pad[pos:pos + k] = src_s[a:b_] - n0          # local src idx
            dstloc[pos:pos + k] = (dst_s[a:b_] - gw * 128).astype(np.float32)
        idx_w = _wrap_idx(src_pad.astype(np.int16))            # [128, T*8]
        # one dstloc column per (tile, window) span
        dcols = np.full((ncols, 128), -1.0, np.float32)
        for t in range(T_total):
            tsl = slice(t * 128, (t + 1) * 128)
            wv = wstream[tsl]
            dv = dstloc[tsl]
            for (w, col) in tile_spans[t]:
                dcols[col] = np.where(wv == w, dv, -1.0)
        dst_t = np.ascontiguousarray(dcols.T)                  # [128, ncols]

        deg_nm = np.ascontiguousarray(
            deg[n0:n0 + NPC].reshape(NW, 128).T)               # [128, 32]

        gidx = (sel[c * GPC:(c + 1) * GPC] +
                np.arange(c * GPC, (c + 1) * GPC)[:, None] * NG - n0)  # local
        gidx_w = gidx.reshape(1, 16).astype(np.float32)        # [1, 16]

        m = {
            "lasth": np.ascontiguousarray(
                lastf[c * GPC:(c + 1) * GPC].reshape(GPC * S, E)),
            "firsth": np.ascontiguousarray(
                firstf[c * GPC:(c + 1) * GPC].reshape(GPC * S, E)),
            "xT": np.ascontiguousarray(
                xT.reshape(3, 128, N)[:, :, n0:n0 + NPC].transpose(1, 0, 2)
            ).reshape(128, 3 * NPC),
            "eidx": idx_w, "dstloc": dst_t, "deg": deg_nm,
            "vecs": vecs, "brow": brow, "gidx": gidx_w,
            "w1": w_bf["w1"].reshape(128, 3 * D),
            "w2": w_bf["w2"].reshape(128, 3 * D),
            "wc": w_bf["wc"].reshape(128, 3 * D),
            "wp1": w_bf["wp1"].reshape(128, 3 * D),
            "wp2": w_bf["wp2"].reshape(128, 3 * D),
            "wcat": wcat.reshape(128, 6 * E),
            "wout": wout.reshape(128, 6 * C),
        }
        in_maps.append(m)
    meta = (tuple(int(e) for e in ec),)
    return in_maps, meta


# ---------------------------------------------------------------- device
def build(meta, rep=1, taps=(), stage=99):
    ec = np.asarray(meta[0], np.int64)
    NWG = len(ec)
    estart = np.zeros(NWG + 1, np.int64)
    np.cumsum(ec, out=estart[1:])
    Etot = int(estart[-1])
    T_total = cdiv(Etot, 128)
    EPAD = T_total * 128
    wstream = np.repeat(np.arange(NWG), ec)
    wstream = np.concatenate([wstream, np.full(EPAD - Etot, -1, np.int64)])
    ncols = 0
    tile_spans = []
    for t in range(T_total):
        ws = np.unique(wstream[t * 128:(t + 1) * 128])
        ws = [int(w) for w in ws if w >= 0]
        tile_spans.append([(w, ncols + i) for i, w in enumerate(ws)])
        ncols += len(ws)
    wt_first = {w: estart[w] // 128 for w in range(NWG)}
    wt_last = {w: (estart[w + 1] - 1) // 128 for w in range(NWG)}

    nc = bacc.Bacc("TRN2")
    I = {}
    I["lasth"] = nc.dram_tensor("lasth", [GPC * S, E], bf16, kind="ExternalInput")
    I["firsth"] = nc.dram_tensor("firsth", [GPC * S, E], bf16, kind="ExternalInput")
    I["xT"] = nc.dram_tensor("xT", [128, 3 * NPC], bf16, kind="ExternalInput")
    I["eidx"] = nc.dram_tensor("eidx", [128, T_total * 8], i16, kind="ExternalInput")
    I["dstloc"] = nc.dram_tensor("dstloc", [128, ncols], fp32, kind="ExternalInput")
    I["deg"] = nc.dram_tensor("deg", [128, NW], fp32, kind="ExternalInput")
    I["vecs"] = nc.dram_tensor("vecs", [128, 72], fp32, kind="ExternalInput")
    I["brow"] = nc.dram_tensor("brow", [1, 512], fp32, kind="ExternalInput")
    I["gidx"] = nc.dram_tensor("gidx", [1, 16], fp32, kind="ExternalInput")
    for w in ("w1", "w2", "wc", "wp1", "wp2"):
        I[w] = nc.dram_tensor(w, [128, 3 * D], bf16, kind="ExternalInput")
    I["wcat"] = nc.dram_tensor("wcat", [128, 6 * E], bf16, kind="ExternalInput")
    I["wout"] = nc.dram_tensor("wout", [128, 6 * C], fp32, kind="ExternalInput")
    outT = nc.dram_tensor("outT", [C, GPC], fp32, kind="ExternalOutput")
    tap_outs = {}

    grp = [list(range(NCORES))]

    with tile.TileContext(nc) as tc, ExitStack() as ctx:
        const = ctx.enter_context(tc.tile_pool(name="const", bufs=1))
        big = ctx.enter_context(tc.tile_pool(name="big", bufs=2))
        gpool = ctx.enter_context(tc.tile_pool(name="gath", bufs=2))
        spool = ctx.enter_context(tc.tile_pool(name="small", bufs=2))
        selp = ctx.enter_context(tc.tile_pool(name="sel", bufs=8))
        hspool = ctx.enter_context(tc.tile_pool(name="hs", bufs=8))
        cinp = ctx.enter_context(tc.tile_pool(name="cin", bufs=1))
        mmps = ctx.enter_context(tc.tile_pool(name="mmps", bufs=2, space="PSUM"))
        cvps = ctx.enter_context(tc.tile_pool(name="cvps", bufs=2, space="PSUM"))
        trps = ctx.enter_context(tc.tile_pool(name="trps", bufs=2, space="PSUM"))
        typs = ctx.enter_context(tc.tile_pool(name="typs", bufs=2, space="PSUM"))
        dram = ctx.enter_context(tc.tile_pool(name="dram", bufs=1, space="DRAM"))

        # ---------------- constants
        iota = const.tile([128, 128], bf16)
        nc.gpsimd.iota(iota[:], pattern=[[1, 128]], base=0, channel_multiplier=0,
                       allow_small_or_imprecise_dtypes=True)
        pidx = const.tile([128, 1], fp32)  # partition index column
        nc.gpsimd.iota(pidx[:], pattern=[[0, 1]], base=0, channel_multiplier=1,
                       allow_small_or_imprecise_dtypes=True)
        ident = const.tile([128, 128], bf16)
        nc.vector.tensor_scalar(ident[:], iota[:], pidx[:], None, ALU.is_equal)

        # input DMAs ordered by when compute needs them: xT (L1) first, then
        # the small weights; the big conv-only index/selector tables load
        # later (during the L2/AG2 window).
        xT_t = big.tile([128, 3, NPC], bf16, tag="big")
        nc.sync.dma_start(xT_t[:], I["xT"][:].rearrange("p (k n) -> p k n", k=3))
        W = {}
        for w in ("w1", "w2", "wc", "wp1", "wp2"):
            W[w] = const.tile([128, 3, D], bf16, name=f"W_{w}", tag=f"W_{w}")
            nc.sync.dma_start(W[w][:], I[w][:].rearrange("p (k m) -> p k m", k=3))
        deg_t = const.tile([128, NW], fp32)
        nc.sync.dma_start(deg_t[:], I["deg"][:])
        vecs = const.tile([128, 72], fp32)
        nc.sync.dma_start(vecs[:], I["vecs"][:])
        brow = const.tile([1, 512], fp32)
        nc.sync.dma_start(brow[:], I["brow"][:])
        bconv_f = const.tile([128, D], fp32)
        nc.gpsimd.partition_broadcast(bconv_f[:], brow[0:1, 0:D])
        bconv_bc = const.tile([128, D], bf16)
        nc.vector.tensor_copy(bconv_bc[:], bconv_f[:])
        ones8 = const.tile([1, GPC], fp32)
        nc.vector.memset(ones8[:], 1.0)
        onescol = const.tile([128, 1], bf16)
        nc.vector.memset(onescol[:], 1.0)

        # vec column offsets
        VO = {}
        off = 0
        for name, w_ in [("b1", 3), ("b2", 3), ("bp1", 3), ("bp2", 3), ("bcat", 6),
                         ("g0", 3), ("be0", 3), ("g1", 3), ("be1", 3),
                         ("g4", 3), ("be4", 3), ("g5", 3), ("be5", 3),
                         ("g6", 3), ("be6", 3),
                         ("gc0", 6), ("bc0", 6), ("gc1", 6), ("bc1", 6)]:
            VO[name] = (off, w_)
            off += w_
        def vcol(name):
            o, w_ = VO[name]
            return vecs[:, o:o + w_]

        # deg^-1/2
        dinv = const.tile([128, NW], fp32)
        nc.scalar.sqrt(dinv[:], deg_t[:])
        nc.vector.reciprocal(dinv[:], dinv[:])

        def tap(name, ap):
            if name not in taps:
                return
            t_ = nc.dram_tensor(f"tap_{name}", list(ap.shape), ap.dtype,
                                kind="ExternalOutput")
            tap_outs[name] = t_
            nc.sync.dma_start(t_[:], ap)

        def finish_early(src_ap, width=GPC):
            fin0 = spool.tile([C, GPC], fp32, name="fin0", tag="fin")
            nc.vector.memset(fin0[:], 0.0)
            nc.vector.tensor_scalar(fin0[:, 0:width], src_ap, 1.0, None, ALU.mult)
            nc.sync.dma_start(outT[:], fin0[:])

        for _rep in range(rep):
            # ---------------- helpers
            def _combine_bn_raw(raw, st, m_chunks):
                """raw [128, m, NCH, 6] bn_stats outputs -> st [128, m, 2]
                holding (sum/256, sumsq/256) per feature."""
                Ev = raw[:, :, :, 1]
                Vv = raw[:, :, :, 2]
                Ov = raw[:, :, :, 4]
                Wv = raw[:, :, :, 5]
                e2 = spool.tile([128, m_chunks, NCH], fp32, name="e2", tag="cb_e2")
                o2 = spool.tile([128, m_chunks, NCH], fp32, name="o2", tag="cb_o2")
                vv = spool.tile([128, m_chunks, NCH], fp32, name="vv", tag="cb_vv")
                ss = spool.tile([128, m_chunks, NCH], fp32, name="ss", tag="cb_ss")
                nc.vector.tensor_mul(e2[:], Ev, Ev)
                nc.vector.tensor_mul(o2[:], Ov, Ov)
                nc.vector.tensor_add(e2[:], e2[:], o2[:])
                nc.vector.tensor_tensor(vv[:], Vv, Wv, ALU.add)
                nc.vector.scalar_tensor_tensor(
                    vv[:], vv[:], 1.0 / 256, e2[:], op0=ALU.mult, op1=ALU.add)
                nc.vector.tensor_tensor(ss[:], Ev, Ov, ALU.add)
                nc.vector.reduce_sum(st[:, :, 0:1], ss[:], axis=AX.X)
                nc.vector.reduce_sum(st[:, :, 1:2], vv[:], axis=AX.X)

            def bn_stats_pack(src_view, m_chunks, tag):
                """src_view(m) -> [128, NCH, NCOL] bf16 view; returns
                [128, m_chunks*2] (sum/256, sumsq/256) stat tile."""
                raw = spool.tile([128, m_chunks, NCH, 6], fp32, tag=tag + "_bs")
                for m in range(m_chunks):
                    sv = src_view(m)
                    for j in range(NCH):
                        nc.vector.bn_stats(raw[:, m, j:j + 1, :], sv[:, j, :])
                st = spool.tile([128, m_chunks, 2], fp32, tag=tag + "_st")
                _combine_bn_raw(raw, st, m_chunks)
                return st

            def ag_exchange(st_ap, width, tag):
                """AllGather the [128, width] stat tile; sum over ranks."""
                cin = dram.tile([128, width], fp32, tag=tag + "_ci")
                cout = dram.tile([NCORES * 128, width], fp32, tag=tag + "_co")
                nc.sync.dma_start(cin[:], st_ap)
                nc.gpsimd.collective_compute(
                    "AllGather", ALU.bypass, replica_groups=grp,
                    ins=[cin[:]], outs=[cout[:]])
                allsb = spool.tile([128, NCORES, width], fp32, tag=tag + "_as")
                nc.sync.dma_start(
                    allsb[:], cout[:].rearrange("(r p) w -> p r w", p=128))
                red = spool.tile([128, width], fp32, tag=tag + "_rd")
                nc.vector.tensor_tensor(red[:], allsb[:, 0], allsb[:, 1], ALU.add)
                for r in range(2, NCORES):
                    nc.vector.tensor_tensor(red[:], red[:], allsb[:, r], ALU.add)
                return red

            def bn_coeffs(red, m_chunks, count, gname, bname, tag):
                """red [128, 2*m]: per-feature sum/sumsq -> gp, bp [128, m]"""
                gp = spool.tile([128, m_chunks], fp32, tag=tag + "_gp")
                bp = spool.tile([128, m_chunks], fp32, tag=tag + "_bp")
                mu = spool.tile([128, m_chunks], fp32, tag=tag + "_mu")
                var = spool.tile([128, m_chunks], fp32, tag=tag + "_va")
                inv_n = 1.0 / count
                sview = red.rearrange("p (m two) -> p m two", two=2)
                nc.vector.tensor_scalar(mu[:], sview[:, :, 0], inv_n, None, ALU.mult)
                nc.vector.tensor_scalar(var[:], sview[:, :, 1], inv_n, None, ALU.mult)
                musq = spool.tile([128, m_chunks], fp32, tag=tag + "_ms")
                nc.vector.tensor_mul(musq[:], mu[:], mu[:])
                nc.vector.tensor_tensor(var[:], var[:], musq[:], ALU.subtract)
                nc.vector.tensor_scalar(var[:], var[:], EPS, None, ALU.add)
                nc.scalar.sqrt(var[:], var[:])
                nc.vector.reciprocal(var[:], var[:])          # 1/sigma
                nc.vector.tensor_mul(gp[:], vcol(gname), var[:])
                nc.vector.tensor_mul(bp[:], gp[:], mu[:])
                nc.vector.tensor_tensor(bp[:], vcol(bname), bp[:], ALU.subtract)
                return gp, bp

            def bn_apply(x_t, gp, bp, m_chunks, width):
                for m in range(m_chunks):
                    nc.vector.tensor_scalar(
                        x_t[:, m, :width], x_t[:, m, :width],
                        gp[:, m:m + 1], bp[:, m:m + 1], ALU.mult, ALU.add)

            def mlp_layer(x_t, w_t, bias_col, tag):
                """x_t [128,3,NPC] bf16 -> relu(x@W + b) bf16 [128,3,NPC] + stats
                (bn_stats interleaved per chunk so it overlaps the matmuls)"""
                out = big.tile([128, 3, NPC], bf16, tag="big")
                raw = spool.tile([128, 3, NCH, 6], fp32, tag=tag + "_bs")
                for m in range(3):
                    for j in range(NCH):
                        sl = slice(j * NCOL, (j + 1) * NCOL)
                        ps = mmps.tile([128, NCOL], fp32, tag="mm")
                        for k in range(3):
                            nc.tensor.matmul(
                                ps[:], lhsT=w_t[:, k, m * 128:(m + 1) * 128],
                                rhs=x_t[:, k, sl], start=(k == 0), stop=(k == 2))
                        nc.scalar.activation(out[:, m, sl], ps[:], AF.Relu,
                                             bias=bias_col[:, m:m + 1])
                        nc.vector.bn_stats(raw[:, m, j:j + 1, :], out[:, m, sl])
                st = spool.tile([128, 3, 2], fp32, tag=tag + "_st")
                _combine_bn_raw(raw, st, 3)
                return out, st

            # ---------------- sentence branch (first: fills the input-DMA
            # phase and the AG1/AG2 stall windows)
            HsT = spool.tile([128, 6, GPC], fp32, tag="HsT")
            for b in range(GPC):
                ps_ht = typs.tile([128, GPC], fp32, tag="tiny")
                hts = []
                for hsrc in (I["lasth"], I["firsth"]):
                    for sc in range(4):
                        ht = hspool.tile([128, E], bf16, name="ht", tag="ht")
                        nc.sync.dma_start(
                            ht[:], hsrc[b * S + sc * 128:b * S + (sc + 1) * 128, :])
                        hts.append(ht)
                for m in range(6):
                    for i, ht in enumerate(hts):
                        nc.tensor.matmul(
                            ps_ht[:, m:m + 1],
                            lhsT=ht[:, m * 128:(m + 1) * 128],
                            rhs=onescol[:],
                            start=(i == 0), stop=(i == 7))
                nc.vector.tensor_scalar(
                    HsT[:, :, b], ps_ht[:, 0:6],
                    1.0 / (2 * S), None, ALU.mult)
            tap("hsT", HsT[:])
            # H_sent halves of the tail BN stats, computed early
            stt = spool.tile([128, 24], fp32, tag="stt")
            for m in range(6):
                nc.vector.reduce_sum(stt[:, 12 + 2 * m:13 + 2 * m], HsT[:, m, :],
                                     axis=AX.X)
                sqh = spool.tile([128, GPC], fp32, name="sqh", tag="ttsq")
                nc.scalar.square(sqh[:], HsT[:, m, :])
                nc.vector.reduce_sum(stt[:, 13 + 2 * m:14 + 2 * m], sqh[:],
                                     axis=AX.X)

            # ---------------- pre-MLPs
            x1, st1 = mlp_layer(xT_t, W["w1"], vcol("b1"), "l1")
            red1 = ag_exchange(st1[:], 6, "ar1")
            gp1, bp1 = bn_coeffs(red1, 3, N // 256, "g0", "be0", "bn1")
            bn_apply(x1, gp1, bp1, 3, NPC)
            tap("x1", x1[:])

            x2, st2 = mlp_layer(x1, W["w2"], vcol("b2"), "l2")
            red2 = ag_exchange(st2[:], 6, "ar2")
            gp2, bp2 = bn_coeffs(red2, 3, N // 256, "g1", "be1", "bn2")
            bn_apply(x2, gp2, bp2, 3, NPC)
            tap("x2", x2[:])
            if stage <= 2:
                finish_early(x2[0:C, 0, 0:GPC])
                continue

            # conv-only tables: loaded here so their DMAs drain during the
            # L2/AG2 window instead of delaying the first L1 matmul
            idx_t = const.tile([128, T_total * 8], i16)
            nc.sync.dma_start(idx_t[:], I["eidx"][:])
            dst_t = const.tile([128, ncols], fp32)
            nc.sync.dma_start(dst_t[:], I["dstloc"][:])
            gidx_t = const.tile([1, 16], fp32)
            nc.sync.dma_start(gidx_t[:], I["gidx"][:])
            gidx_bc = const.tile([128, 16], fp32)
            nc.gpsimd.partition_broadcast(gidx_bc[:], gidx_t[0:1, :])
            wcat = const.tile([128, 6, E], bf16)
            nc.sync.dma_start(wcat[:],
                              I["wcat"][:].rearrange("p (k m) -> p k m", k=6))
            wout = const.tile([128, 6, C], fp32)
            nc.sync.dma_start(wout[:],
                              I["wout"][:].rearrange("p (k m) -> p k m", k=6))

            # ---------------- z = x2 @ wc ; y = z * dinv (node-major bf16)
            y_slice = dram.tile([NPC, D], bf16, tag="y_slice")
            z_sb = big.tile([128, 3, NPC], bf16, tag="big")
            for m in range(3):
                for j in range(NCH):
                    sl = slice(j * NCOL, (j + 1) * NCOL)
                    ps = mmps.tile([128, NCOL], fp32, tag="mm")
                    for k in range(3):
                        nc.tensor.matmul(
                            ps[:], lhsT=W["wc"][:, k, m * 128:(m + 1) * 128],
                            rhs=x2[:, k, sl], start=(k == 0), stop=(k == 2))
                    nc.scalar.activation(z_sb[:, m, sl], ps[:], AF.Copy)
            for w in range(NW):
                ywin = spool.tile([128, D], bf16, tag="ywin")
                for m in range(3):
                    trp = trps.tile([128, 128], bf16, tag="tr")
                    nc.tensor.transpose(
                        trp[:], z_sb[:, m, w * 128:(w + 1) * 128], ident[:])
                    nc.vector.tensor_scalar(
                        ywin[:, m * 128:(m + 1) * 128], trp[:],
                        dinv[:, w:w + 1], None, ALU.mult)
                nc.sync.dma_start(y_slice[w * 128:(w + 1) * 128, :], ywin[:])

            # ---------------- conv: per-core partial sums over ALL 256 global
            # dst windows (gathering from the LOCAL y_slice only), written
            # bf16 to pagg, then summed+sharded with one ReduceScatter.
            pagg = dram.tile([N, D], bf16, tag="pagg")
            ps_win = {}

            def partial_flush(gw):
                ps_c = ps_win.pop(gw)
                pb = spool.tile([128, D], bf16, name="pb", tag="pb")
                nc.scalar.activation(pb[:], ps_c[:], AF.Copy)
                nc.sync.dma_start(pagg[gw * 128:(gw + 1) * 128, :], pb[:])

            t = 0
            while t < T_total:
                cn = min(GCHUNK, T_total - t)
                gt = gpool.tile([128, GCHUNK * D], bf16, tag="g")
                nc.gpsimd.dma_gather(
                    out_ap=gt[:, :cn * D].rearrange("p (t f) -> p t f", f=D),
                    in_ap=y_slice[:],
                    idxs_ap=idx_t[:, t * 8:(t + cn) * 8],
                    num_idxs=cn * 128, num_idxs_reg=cn * 128, elem_size=D)
                gv = gt[:, :cn * D].rearrange("p (t f) -> p t f", f=D)
                for tl in range(cn):
                    tg = t + tl
                    for (w, col) in tile_spans[tg]:
                        if tg == wt_first[w]:
                            ps_win[w] = cvps.tile([128, D], fp32, name="ps_c",
                                                  tag="cv")
                        sel = selp.tile([128, 128], bf16, tag="sel")
                        nc.vector.tensor_scalar(
                            sel[:], iota[:], dst_t[:, col:col + 1],
                            None, ALU.is_equal)
                        last = (tg == wt_last[w])
                        nc.tensor.matmul(ps_win[w][:], lhsT=sel[:],
                                         rhs=gv[:, tl, :],
                                         start=(tg == wt_first[w]), stop=last)
                        if last:
                            partial_flush(w)
                t += cn

            # ysb does not depend on the ReduceScatter: load it first
            ysb = cinp.tile([128, NW, D], bf16, tag="ysb")
            nc.sync.dma_start(
                ysb[:], y_slice[:].rearrange("(w p) f -> p w f", p=128))

            conv_in = dram.tile([NPC, D], bf16, tag="conv_in")
            nc.gpsimd.collective_compute(
                "ReduceScatter", ALU.add, replica_groups=grp,
                ins=[pagg[:]], outs=[conv_in[:]])
            if stage <= 3:
                finish_early(HsT[0:C, 0, 0:GPC])
                continue

            # ---------------- own-shard finalize: scale/bias/relu/transpose
            convT = big.tile([128, 3, NPC], bf16, tag="big")
            cin_sb = cinp.tile([128, NW, D], bf16, tag="cin")
            nc.sync.dma_start(
                cin_sb[:], conv_in[:].rearrange("(w p) f -> p w f", p=128))
            raw4 = spool.tile([128, 3, NCH, 6], fp32, tag="l4_bs")
            for w in range(NW):
                # fold the self-loop message y[n] into the aggregate
                agg = spool.tile([128, D], bf16, name="agg", tag="agg")
                nc.vector.tensor_add(agg[:], cin_sb[:, w, :], ysb[:, w, :])
                winf = spool.tile([128, D], bf16, tag="winf")
                nc.vector.scalar_tensor_tensor(
                    winf[:], agg[:], dinv[:, w:w + 1], bconv_bc[:],
                    op0=ALU.mult, op1=ALU.add)
                winb = spool.tile([128, D], bf16, tag="winb")
                nc.scalar.activation(winb[:], winf[:], AF.Relu)
                for m in range(3):
                    trp = trps.tile([128, 128], bf16, tag="tr")
                    nc.tensor.transpose(trp[:], winb[:, m * 128:(m + 1) * 128],
                                        ident[:])
                    nc.vector.tensor_copy(convT[:, m, w * 128:(w + 1) * 128],
                                          trp[:])
                if w % 4 == 3:
                    j = w // 4
                    for m in range(3):
                        nc.vector.bn_stats(raw4[:, m, j:j + 1, :],
                                           convT[:, m, j * NCOL:(j + 1) * NCOL])

            st4 = spool.tile([128, 3, 2], fp32, tag="st4")
            _combine_bn_raw(raw4, st4, 3)
            red4 = ag_exchange(st4[:], 6, "ar4")
            gp4, bp4 = bn_coeffs(red4, 3, N // 256, "g4", "be4", "bn4")
            bn_apply(convT, gp4, bp4, 3, NPC)
            tap("convT", convT[:])
            if stage <= 4:
                finish_early(convT[0:C, 0, 0:GPC])
                continue

            # ---------------- post MLPs
            p1, st5 = mlp_layer(convT, W["wp1"], vcol("bp1"), "l5")
            red5 = ag_exchange(st5[:], 6, "ar5")
            gp5, bp5 = bn_coeffs(red5, 3, N // 256, "g5", "be5", "bn5")
            bn_apply(p1, gp5, bp5, 3, NPC)

            # post2 (pre-BN, stays in SBUF): stats collected per chunk; the 16
            # masked rows are extracted inline with an accumulating one-hot
            # matmul over the node-major transposed tiles.
            raw6 = spool.tile([128, 3, NCH, 6], fp32, tag="p2_bs")
            ps16 = cvps.tile([128, D], fp32, name="ps16", tag="cv")
            for j in range(NCH):
                sl = slice(j * NCOL, (j + 1) * NCOL)
                p2c = spool.tile([128, 3, NCOL], bf16, tag="p2c")
                for m in range(3):
                    ps = mmps.tile([128, NCOL], fp32, tag="mm")
                    for k in range(3):
                        nc.tensor.matmul(
                            ps[:], lhsT=W["wp2"][:, k, m * 128:(m + 1) * 128],
                            rhs=p1[:, k, sl], start=(k == 0), stop=(k == 2))
                    nc.scalar.activation(p2c[:, m, :], ps[:], AF.Relu,
                                         bias=vcol("bp2")[:, m:m + 1])
                    nc.vector.bn_stats(raw6[:, m, j:j + 1, :], p2c[:, m, :])
                for nb in range(NCOL // 128):
                    rmw = spool.tile([128, D], bf16, tag="rmw")
                    for m in range(3):
                        trp = trps.tile([128, 128], bf16, tag="tr")
                        nc.tensor.transpose(
                            trp[:], p2c[:, m, nb * 128:(nb + 1) * 128], ident[:])
                        nc.vector.tensor_copy(rmw[:, m * 128:(m + 1) * 128], trp[:])
                    # one-hot columns for masked nodes in this 128-row window
                    blk = j * NCOL + nb * 128
                    ohs = spool.tile([128, 16], fp32, name="ohs", tag="ohs")
                    nc.vector.tensor_scalar(ohs[:], gidx_bc[:], float(blk),
                                            None, ALU.subtract)
                    oh = spool.tile([128, 16], bf16, name="oh", tag="oh")
                    nc.vector.tensor_scalar(oh[:], ohs[:], pidx[:], None,
                                            ALU.is_equal)
                    nc.tensor.matmul(ps16[0:16, :], lhsT=oh[:], rhs=rmw[:],
                                     start=(blk == 0),
                                     stop=(blk == NPC - 128))
            # combine raw6 -> st6 [128, 3, 2] (sum/256, sumsq/256)
            st6 = spool.tile([128, 3, 2], fp32, tag="st6")
            _combine_bn_raw(raw6, st6, 3)
            red6 = ag_exchange(st6[:], 6, "ar6")
            gp6, bp6 = bn_coeffs(red6, 3, N // 256, "g6", "be6", "bn6")
            if stage <= 5:
                finish_early(red6[0:C, 0:6], width=6)
                continue

            # ---------------- masked rows -> flT [128, 3, 16] bf16 (BN6'd)
            gth = spool.tile([128, D], bf16, tag="gth")
            nc.scalar.activation(gth[0:16, :], ps16[0:16, :], AF.Copy)
            flT = spool.tile([128, 3, 16], bf16, tag="flT")
            for m in range(3):
                trp_full = trps.tile([128, 128], bf16, tag="tr")
                trp = trp_full[:, 0:16]
                nc.tensor.matmul(trp, lhsT=gth[0:16, m * 128:(m + 1) * 128],
                                 rhs=ident[0:16, 0:16], is_transpose=True)
                nc.vector.tensor_scalar(flT[:, m, :], trp,
                                        gp6[:, m:m + 1], bp6[:, m:m + 1],
                                        ALU.mult, ALU.add)
            tap("flT", flT[:])

            # ---------------- tail: outc, H_sent BN, att, out
            outcT = spool.tile([128, 6, GPC], fp32, tag="outcT")
            for m in range(6):
                ps_o = typs.tile([128, GPC], fp32, tag="tiny")
                for k in range(6):
                    kc, kj = k % 3, k // 3
                    nc.tensor.matmul(
                        ps_o[:], lhsT=wcat[:, k, m * 128:(m + 1) * 128],
                        rhs=flT[:, kc, kj::2], start=(k == 0), stop=(k == 5))
                nc.scalar.activation(outcT[:, m, :], ps_o[:], AF.Relu,
                                     bias=vcol("bcat")[:, m:m + 1])
            for m in range(6):
                nc.vector.reduce_sum(stt[:, 2 * m:2 * m + 1], outcT[:, m, :], axis=AX.X)
                sq = spool.tile([128, GPC], fp32, tag="ttsq")
                nc.scalar.square(sq[:], outcT[:, m, :])
                nc.vector.reduce_sum(stt[:, 2 * m + 1:2 * m + 2], sq[:], axis=AX.X)
            redt = ag_exchange(stt[:], 24, "art")
            gpc_, bpc_ = bn_coeffs(redt[:, 0:12], 6, B, "gc0", "bc0", "bnc")
            gph, bph = bn_coeffs(redt[:, 12:24], 6, B, "gc1", "bc1", "bnh")
            attT = spool.tile([128, 6, GPC], fp32, tag="attT")
            for m in range(6):
                nc.vector.tensor_scalar(attT[:, m, :], HsT[:, m, :],
                                        gph[:, m:m + 1], bph[:, m:m + 1],
                                        ALU.mult, ALU.add)
                nc.vector.tensor_scalar(outcT[:, m, :], outcT[:, m, :],
                                        gpc_[:, m:m + 1], bpc_[:, m:m + 1],
                                        ALU.mult, ALU.add)
                nc.vector.tensor_add(attT[:, m, :], attT[:, m, :], outcT[:, m, :])
            ps_ft = typs.tile([128, GPC], fp32, tag="tiny")
            ps_f = ps_ft[0:C, :]
            for k in range(6):
                kc, kj = k % 3, k // 3
                nc.tensor.matmul(ps_f, lhsT=wout[:, k, :], rhs=attT[:, k, :],
                                 start=(k == 0), stop=False)
            nc.tensor.matmul(ps_f, lhsT=brow[0:1, D:D + C], rhs=ones8[:],
                             start=False, stop=True)
            fin = spool.tile([C, GPC], fp32, tag="fin")
            nc.vector.tensor_copy(fin[:], ps_f)
            nc.sync.dma_start(outT[:], fin[:])

    nc.compile()
    return nc, tap_outs


# ---------------------------------------------------------------- entry
_CACHE = {}


def _get_compiled(meta):
    key = meta
    if key not in _CACHE:
        nc, _ = build(meta)
        split_waits(nc)
        _CACHE[key] = nc
    return _CACHE[key]


def kernel(**inputs):
    in_maps, meta = preprocess(inputs)
    nc = _get_compiled(meta)
    from concourse import bass2jax
    results = bass2jax.run_bass_via_pjrt(nc, in_maps, n_cores=NCORES)
    out = np.concatenate([results[c]["outT"].T for c in range(NCORES)], axis=0)
    return out.astype(np.float32)
